# revision 26
# baseline (speedup 1.0000x reference)
"""AZConv2d fused anisotropic conv kernel for Trainium2 (Bass/Tile), v3.

Math (per pixel l, rule r):
  gate = gate_w@x + gate_b; mu = softmax_r(gate)
  v = value_w@x;  geom = geom_w@x + geom_b -> theta, raw_base, raw_hyper (4 each)
  base = softplus(rb)+1e-4; hyper = softplus(rh)+0.1
  iu2 = 1/(base*e^h)^2; is2 = 1/(base*e^-h)^2
  kern(dy,dx) = exp(-(a*dx^2 + b*dy^2 + c2*dx*dy)),
     a = ct^2*iu2 + st^2*is2, b = st^2*iu2+ct^2*is2, c2 = 2*ct*st*(iu2-is2)
  w = mu*kern / (sum_{r,s} mu*kern + 1e-6)
  agg[r,c] = sum_s w[r,s] * v[c, l+delta_s];  out = pw_w @ agg + pw_b

With V0=v, V1=u=v(l-1)+v(l+1), V2=v(l-W)+v(l+W), S=u(l-W)+u(l+W),
T=wd(l-W)-wd(l+W), wd=v(l-1)-v(l+1):
  agg_r = G0*V0 + G1*V1 + G2*V2 + G3*S + G4*T,
  G0=em', G1=em'A, G2=em'B, G3=em'AB*cosh(c2), G4=-em'AB*sinh(c2).

v3: whole-image persistent fields in c-major [128 x-pixels, 64 chan, y]
layout (y-minor keeps every MAC operand unit-stride innermost -> DVE 2x
mode). x-shifts via partition-shifted sbuf->sbuf DMA (no x1/xd ops, no
u/w projections, no halo recompute). MAC [r, c, y] iteration, PE
transposes + pointwise as in the baseline.

Sharding: data-parallel over batch, 1 image per NeuronCore (8 cores).
"""

import math
import sys

for p in ("/opt/trn_rl_repo",):
    if p not in sys.path:
        sys.path.insert(0, p)

import ml_dtypes
import numpy as np

import concourse.bass as bass
import concourse.mybir as mybir
import concourse.tile as tile
from concourse import bacc
from concourse.bass import ds
from concourse.masks import make_identity

F32 = mybir.dt.float32
BF16 = mybir.dt.bfloat16
AF = mybir.ActivationFunctionType
ALU = mybir.AluOpType

B = 8
C = 64
H = 128
W = 128
L = H * W
R = 4
BAND = 8
NBANDS = H // BAND  # 16
EPS = 1e-4
MIN_HYP = 0.1
LN2 = math.log(2.0)


def _projection(nc, pools, x2, rhs0, v, gg):
    """All 128 row projections -> v [128, 64, 130] bf16 (y slot j = row j-1)
    and gg [128, 16, 128] f32 (gate 0:4 | theta 4:8 | rbase 8:12 | rhyp 12:16).
    """
    pps = pools["ps"]
    for half in range(2):
        p0 = 64 * half
        rowbase = 0 if half == 0 else 63

        def xrow(y):
            return x2[p0 : p0 + 64, ds((y - rowbase) * W, W)]

        base = 64 * half
        for g in range(13):  # 13 groups of 5 rows (last group 4) per half
            ra = base + g * 5
            rb = min(ra + 5, base + 64)
            n = rb - ra
            ps = pps.tile([128, 5, 80], F32, tag="psv")
            for j in range(n):
                nc.tensor.matmul(
                    out=ps[:, j, :],
                    lhsT=xrow(ra + j),
                    rhs=rhs0[p0 : p0 + 64, :],
                    start=True,
                    stop=True,
                )
            # transposed copies: psum [y, c] -> sbuf [c, y]
            nc.scalar.activation(
                out=v[:, :, ra + 1 : rb + 1],
                in_=ps[:, 0:n, 0:64].rearrange("p y c -> p c y"),
                func=AF.Copy,
            )
            nc.scalar.activation(
                out=gg[:, :, ra:rb],
                in_=ps[:, 0:n, 64:80].rearrange("p y f -> p f y"),
                func=AF.Copy,
            )


def _geometry(nc, pools, gg, gbias, mbias, cb, G, h):
    """Coefficients G[0..4] (each [128, 4, 128] bf16, y-minor) for quad h
    (rows 32h..32h+31) from gg [128, 16, 128] f32."""
    pg = pools["geo"]
    Y = 32
    yc = slice(32 * h, 32 * h + 32)

    def gt(tag, nf=4, dt=F32):
        return pg.tile([128, nf, Y], dt, tag=tag, name=tag)

    gate = gg[:, 0:4, yc]
    theta = gg[:, 4:8, yc]

    nc.vector.tensor_tensor(
        out=gate, in0=gate, in1=gbias[:, :, None].broadcast_to([128, 4, Y]),
        op=ALU.add,
    )
    nc.vector.tensor_tensor(
        out=gg[:, 4:16, yc],
        in0=gg[:, 4:16, yc],
        in1=mbias[:, :, None].broadcast_to([128, 12, Y]),
        op=ALU.add,
    )

    # trig; ct/st packed as cst = [ct | st]
    cst = gt("cst", 8)
    ct, st = cst[:, 0:4, :], cst[:, 4:8, :]
    sh = gt("sh")
    nc.scalar.activation(out=st, in_=theta, func=AF.Sin)
    nc.scalar.activation(out=sh, in_=theta, func=AF.Sin, scale=0.5)
    sh2 = gt("sh2")
    nc.vector.tensor_tensor(out=sh2, in0=sh, in1=sh, op=ALU.mult)
    nc.vector.tensor_scalar(
        out=ct, in0=sh2, scalar1=-2.0, scalar2=1.0, op0=ALU.mult, op1=ALU.add
    )

    # softmax numerator: em = exp(gate - max_r gate)
    mx = pg.tile([128, 1, Y], F32, tag="mx")
    m01 = pg.tile([128, 1, Y], F32, tag="m01")
    m23 = pg.tile([128, 1, Y], F32, tag="m23")
    nc.vector.tensor_tensor(
        out=m01, in0=gate[:, 0:1, :], in1=gate[:, 1:2, :], op=ALU.max
    )
    nc.vector.tensor_tensor(
        out=m23, in0=gate[:, 2:3, :], in1=gate[:, 3:4, :], op=ALU.max
    )
    nc.vector.tensor_tensor(out=mx, in0=m01, in1=m23, op=ALU.max)
    gsh = gt("gsh")
    nc.vector.tensor_tensor(
        out=gsh, in0=gate, in1=mx.broadcast_to([128, 4, Y]), op=ALU.subtract
    )
    em = gt("em")
    nc.scalar.activation(out=em, in_=gsh, func=AF.Exp)

    # softplus(base|hyper) = ln(1 + exp(.)), then ln(base+eps)
    e8 = gt("e8", 8)
    nc.scalar.activation(out=e8, in_=gg[:, 8:16, yc], func=AF.Exp)
    sp8 = gt("sp8", 8)
    nc.scalar.activation(out=sp8, in_=e8, func=AF.Ln, bias=1.0)
    spb, sph = sp8[:, 0:4, :], sp8[:, 4:8, :]
    lb = gt("lb")
    nc.scalar.activation(out=lb, in_=spb, func=AF.Ln, bias=cb["eps"])

    tpl, tmi = gt("tpl"), gt("tmi")
    nc.vector.tensor_tensor(out=tpl, in0=sph, in1=lb, op=ALU.add)
    nc.vector.tensor_tensor(out=tmi, in0=sph, in1=lb, op=ALU.subtract)
    ii = gt("ii", 8)
    iu2, is2 = ii[:, 0:4, :], ii[:, 4:8, :]
    nc.scalar.activation(out=iu2, in_=tpl, func=AF.Exp, scale=-2.0, bias=cb["mh2n"])
    nc.scalar.activation(out=is2, in_=tmi, func=AF.Exp, scale=2.0, bias=cb["mh2p"])

    sq = gt("sq", 8)
    nc.vector.tensor_tensor(out=sq, in0=cst, in1=cst, op=ALU.mult)
    t12 = gt("t12", 8)
    nc.vector.tensor_tensor(out=t12, in0=sq, in1=ii, op=ALU.mult)
    av = gt("av")
    nc.vector.tensor_tensor(out=av, in0=t12[:, 0:4, :], in1=t12[:, 4:8, :], op=ALU.add)
    ssum, bv = gt("ssum"), gt("bv")
    nc.vector.tensor_tensor(out=ssum, in0=iu2, in1=is2, op=ALU.add)
    nc.vector.tensor_tensor(out=bv, in0=ssum, in1=av, op=ALU.subtract)

    cs, dio, c2h = gt("cs"), gt("dio"), gt("c2h")
    nc.vector.tensor_tensor(out=cs, in0=ct, in1=st, op=ALU.mult)
    nc.vector.tensor_tensor(out=dio, in0=iu2, in1=is2, op=ALU.subtract)
    nc.vector.tensor_tensor(out=c2h, in0=cs, in1=dio, op=ALU.mult)

    Aa, Bb = gt("Aa"), gt("Bb")
    nc.scalar.activation(out=Aa, in_=av, func=AF.Exp, scale=-1.0)
    nc.scalar.activation(out=Bb, in_=bv, func=AF.Exp, scale=-1.0)
    # Corner kernels, overflow-safe: a+b = ssum >= |c2|
    sc2, sc2m = gt("sc2"), gt("sc2m")
    nc.vector.scalar_tensor_tensor(
        out=sc2, in0=c2h, scalar=2.0, in1=ssum, op0=ALU.mult, op1=ALU.add
    )
    nc.vector.scalar_tensor_tensor(
        out=sc2m, in0=c2h, scalar=-2.0, in1=ssum, op0=ALU.mult, op1=ALU.add
    )
    ph, qh = gt("ph"), gt("qh")
    nc.scalar.activation(out=ph, in_=sc2, func=AF.Exp, scale=-1.0, bias=cb["mln2"])
    nc.scalar.activation(out=qh, in_=sc2m, func=AF.Exp, scale=-1.0, bias=cb["mln2"])
    ppqh, pmqh = gt("ppqh"), gt("pmqh")
    nc.vector.tensor_tensor(out=ppqh, in0=ph, in1=qh, op=ALU.add)
    nc.vector.tensor_tensor(out=pmqh, in0=ph, in1=qh, op=ALU.subtract)

    # Sk = 1 + 2(A+B) + 4*(P+Q)/2
    apb, w1, sk = gt("apb"), gt("w1"), gt("sk")
    nc.vector.tensor_tensor(out=apb, in0=Aa, in1=Bb, op=ALU.add)
    nc.vector.scalar_tensor_tensor(
        out=w1, in0=ppqh, scalar=2.0, in1=apb, op0=ALU.mult, op1=ALU.add
    )
    nc.vector.tensor_scalar(
        out=sk, in0=w1, scalar1=2.0, scalar2=1.0, op0=ALU.mult, op1=ALU.add
    )

    ws = gt("ws")
    nc.vector.tensor_tensor(out=ws, in0=em, in1=sk, op=ALU.mult)
    d01 = pg.tile([128, 1, Y], F32, tag="d01", name="d01")
    d23 = pg.tile([128, 1, Y], F32, tag="d23", name="d23")
    Dp = pg.tile([128, 1, Y], F32, tag="Dp", name="Dp")
    nc.vector.tensor_tensor(out=d01, in0=ws[:, 0:1, :], in1=ws[:, 1:2, :], op=ALU.add)
    nc.vector.tensor_tensor(out=d23, in0=ws[:, 2:3, :], in1=ws[:, 3:4, :], op=ALU.add)
    nc.vector.tensor_tensor(out=Dp, in0=d01, in1=d23, op=ALU.add)
    s01 = pg.tile([128, 1, Y], F32, tag="s01", name="s01")
    s23 = pg.tile([128, 1, Y], F32, tag="s23", name="s23")
    Smu = pg.tile([128, 1, Y], F32, tag="Smu", name="Smu")
    nc.vector.tensor_tensor(out=s01, in0=em[:, 0:1, :], in1=em[:, 1:2, :], op=ALU.add)
    nc.vector.tensor_tensor(out=s23, in0=em[:, 2:3, :], in1=em[:, 3:4, :], op=ALU.add)
    nc.vector.tensor_tensor(out=Smu, in0=s01, in1=s23, op=ALU.add)
    D2 = pg.tile([128, 1, Y], F32, tag="D2")
    nc.vector.scalar_tensor_tensor(
        out=D2, in0=Smu, scalar=1e-6, in1=Dp, op0=ALU.mult, op1=ALU.add
    )
    invD = pg.tile([128, 1, Y], F32, tag="invD")
    nc.vector.reciprocal(invD, D2)

    invDb = invD.broadcast_to([128, 4, Y])
    nc.vector.tensor_tensor(out=G[0][:, :, yc], in0=em, in1=invDb, op=ALU.mult)
    em2 = G[0][:, :, yc]
    nc.vector.tensor_tensor(out=G[1][:, :, yc], in0=em2, in1=Aa, op=ALU.mult)
    nc.vector.tensor_tensor(out=G[2][:, :, yc], in0=em2, in1=Bb, op=ALU.mult)
    nc.vector.tensor_tensor(out=G[3][:, :, yc], in0=em2, in1=ppqh, op=ALU.mult)
    nc.vector.tensor_tensor(out=G[4][:, :, yc], in0=em2, in1=pmqh, op=ALU.mult)


def _mac_band(nc, pools, bi, v, u, wd, G, ident, pwt, pwb, out_d):
    """MAC + PE transpose + pointwise for band pair bi (rows y0..y0+15)."""
    pm, ppsT, ppso, pout = pools["mac"], pools["psT"], pools["pso"], pools["out"]
    NY = 2 * BAND
    y0 = bi * NY
    sh = [128, R, 64, NY]  # [x, r, c, y]

    def bf(t, off):  # field [128, 64, 130] -> [r, c, y] view (bcast r)
        return t[:, None, :, y0 + off : y0 + off + NY].broadcast_to(sh)

    def bg(t):  # coeff [128, 4, 128] -> [r, c, y] view (bcast c)
        return t[:, :, None, y0 : y0 + NY].broadcast_to(sh)

    # v2/vs/vt for this band pair (c-major, y-minor views of whole fields)
    v2 = pm.tile([128, 64, NY], BF16, tag="v2")
    vs = pm.tile([128, 64, NY], BF16, tag="vs")
    vt = pm.tile([128, 64, NY], BF16, tag="vt")
    nc.vector.tensor_tensor(
        out=v2, in0=v[:, :, y0 : y0 + NY], in1=v[:, :, y0 + 2 : y0 + NY + 2],
        op=ALU.add,
    )
    nc.vector.tensor_tensor(
        out=vs, in0=u[:, :, y0 : y0 + NY], in1=u[:, :, y0 + 2 : y0 + NY + 2],
        op=ALU.add,
    )
    nc.vector.tensor_tensor(
        out=vt, in0=wd[:, :, y0 : y0 + NY], in1=wd[:, :, y0 + 2 : y0 + NY + 2],
        op=ALU.subtract,
    )

    def bb(t):  # band field [128, 64, 16] -> [r, c, y]
        return t[:, None, :, :].broadcast_to(sh)

    agg = pm.tile(sh, BF16, tag="agg")
    tA = pm.tile(sh, BF16, tag="tA")
    tB = pm.tile(sh, BF16, tag="tB")
    nc.vector.tensor_tensor(out=tA, in0=bf(v, 1), in1=bg(G[0]), op=ALU.mult)
    nc.vector.tensor_tensor(out=tB, in0=bf(u, 1), in1=bg(G[1]), op=ALU.mult)
    nc.vector.tensor_tensor(out=agg, in0=tA, in1=tB, op=ALU.add)
    nc.vector.tensor_tensor(out=tA, in0=bb(v2), in1=bg(G[2]), op=ALU.mult)
    nc.vector.tensor_tensor(out=tB, in0=bb(vs), in1=bg(G[3]), op=ALU.mult)
    nc.vector.tensor_tensor(out=agg, in0=agg, in1=tA, op=ALU.add)
    nc.vector.tensor_tensor(out=tA, in0=bb(vt), in1=bg(G[4]), op=ALU.mult)
    nc.vector.tensor_tensor(out=agg, in0=agg, in1=tB, op=ALU.add)
    nc.vector.tensor_tensor(out=agg, in0=agg, in1=tA, op=ALU.add)

    # transpose [pixel, (r c)] -> [(r c), pixel] via PE, half (128 rc) at a time
    aggT = pm.tile([128, 2, NY, 128], BF16, tag="aggT")
    for hg in range(2):
        for grp in range(NY // 4):
            psT = ppsT.tile([128, 4, 128], BF16, tag="psT")
            for yy in range(4):
                yr = grp * 4 + yy
                nc.tensor.transpose(
                    out=psT[:, yy, :],
                    in_=agg[:, 2 * hg : 2 * hg + 2, :, yr].rearrange(
                        "p a b -> p (a b)"
                    ),
                    identity=ident,
                )
            nc.scalar.activation(
                out=aggT[:, hg, grp * 4 : grp * 4 + 4, :], in_=psT, func=AF.Copy
            )

    # out = pw @ agg + pw_b (c-major), 512 pixels per psum chunk
    for ch in range(NY * W // 512):
        ps_o = ppso.tile([64, 512], F32, tag="pso")
        for hg in range(2):
            nc.tensor.matmul(
                out=ps_o,
                lhsT=pwt[:, hg, :],
                rhs=aggT[:, hg, ch * 4 : ch * 4 + 4, :].rearrange("p a b -> p (a b)"),
                start=(hg == 0),
                stop=(hg == 1),
            )
        osb = pout.tile([64, 512], F32, tag="osb")
        nc.scalar.activation(out=osb, in_=ps_o, func=AF.Identity, bias=pwb, scale=1.0)
        # alternate issuing engine so output DMAs use two hardware queues
        eng = nc.sync if (bi + ch) % 2 == 0 else nc.scalar
        eng.dma_start(out=out_d[:, ds(y0 * W + ch * 512, 512)], in_=osb)


def build_nc():
    nc = bacc.Bacc("TRN2")
    x_d = nc.dram_tensor("x", [C, L], F32, kind="ExternalInput").ap()
    sh_d = nc.dram_tensor("shiftm", [128, 256], BF16, kind="ExternalInput").ap()
    rhs0_d = nc.dram_tensor("rhs0", [C, 80], BF16, kind="ExternalInput").ap()
    pwt_d = nc.dram_tensor("pw_t2", [128, 2, 64], BF16, kind="ExternalInput").ap()
    gb_d = nc.dram_tensor("gate_b", [R], F32, kind="ExternalInput").ap()
    mb_d = nc.dram_tensor("geom_b", [12], F32, kind="ExternalInput").ap()
    pb_d = nc.dram_tensor("pw_b", [C], F32, kind="ExternalInput").ap()
    out_d = nc.dram_tensor("out", [C, L], F32, kind="ExternalOutput").ap()

    with tile.TileContext(nc) as tc:
        import contextlib

        with contextlib.ExitStack() as ctx:
            persist = ctx.enter_context(tc.tile_pool(name="persist", bufs=1))
            pools = {
                "geo": ctx.enter_context(tc.tile_pool(name="geo", bufs=2)),
                "mac": ctx.enter_context(tc.tile_pool(name="mac", bufs=2)),
                "out": ctx.enter_context(tc.tile_pool(name="out", bufs=2)),
                "psS": ctx.enter_context(
                    tc.tile_pool(name="psS", bufs=2, space="PSUM")
                ),
                "ps": ctx.enter_context(tc.tile_pool(name="ps", bufs=2, space="PSUM")),
                "psT": ctx.enter_context(
                    tc.tile_pool(name="psT", bufs=2, space="PSUM")
                ),
                "pso": ctx.enter_context(
                    tc.tile_pool(name="pso", bufs=2, space="PSUM")
                ),
            }

            x2 = persist.tile([128, 65 * W], BF16)
            rhs0 = persist.tile([128, 80], BF16)
            pwt = persist.tile([128, 2, 64], BF16)
            gbias = persist.tile([128, R], F32)
            mbias = persist.tile([128, 12], F32)
            pwb = persist.tile([64, 1], F32)
            ident = persist.tile([128, 128], BF16)
            make_identity(nc, ident)
            shiftm = persist.tile([128, 256], BF16)
            nc.sync.dma_start(out=shiftm, in_=sh_d)
            v = persist.tile([128, 64, 130], BF16)
            u = persist.tile([128, 64, 130], BF16)
            wd = persist.tile([128, 64, 130], BF16)
            gg = persist.tile([128, 16, 128], F32)
            G = [persist.tile([128, 4, 128], BF16, name=f"G{i}") for i in range(5)]

            cb = {}
            for cname, cval in [
                ("eps", EPS),
                ("mh2n", -2.0 * MIN_HYP),
                ("mh2p", 2.0 * MIN_HYP),
                ("mln2", -LN2),
            ]:
                t = persist.tile([128, 1], F32, tag="cb_" + cname, name="cb_" + cname)
                nc.gpsimd.memset(t, cval)
                cb[cname] = t

            CH = 65 * W // 8
            for c8 in range(8):
                nc.gpsimd.dma_start(
                    out=x2[0:64, ds(c8 * CH, CH)], in_=x_d[:, ds(c8 * CH, CH)]
                )
                nc.gpsimd.dma_start(
                    out=x2[64:128, ds(c8 * CH, CH)],
                    in_=x_d[:, ds(63 * W + c8 * CH, CH)],
                )
            nc.sync.dma_start(out=rhs0[0:64, :], in_=rhs0_d)
            nc.sync.dma_start(out=rhs0[64:128, :], in_=rhs0_d)
            nc.sync.dma_start(out=pwt, in_=pwt_d)
            nc.gpsimd.dma_start(
                out=gbias,
                in_=bass.AP(tensor=gb_d.tensor, offset=0, ap=[[0, 128], [1, R]]),
            )
            nc.gpsimd.dma_start(
                out=mbias,
                in_=bass.AP(tensor=mb_d.tensor, offset=0, ap=[[0, 128], [1, 12]]),
            )
            nc.sync.dma_start(out=pwb, in_=pb_d.rearrange("(c o) -> c o", o=1))

            # projections fill v slots 1..128 (c-major) and gg
            _projection(nc, pools, x2, rhs0, v, gg)
            # y halos
            nc.gpsimd.memset(v[:, :, 0:1], 0.0)
            nc.gpsimd.memset(v[:, :, 129:130], 0.0)

            # x-shifts via PE: u = Tri @ v (pair sum), wd = WdT^T @ v (pair
            # diff); Tri/WdT are constant 128x128 0/±1 matrices acting on the
            # pixel (partition) index. Edge rows are zero automatically.
            # y-blocked so each block only depends on the projections of its
            # own rows (pipelines with the projection stage)
            psS = pools["psS"]
            ybl = [0, 8, 16, 24, 32, 40, 48, 56, 64, 72, 80, 88, 96, 104, 112,
                   120, 128, 130]
            for which, mat, dst in ((0, shiftm[:, 0:128], u),
                                    (1, shiftm[:, 128:256], wd)):
                for k in range(len(ybl) - 1):
                    ya, yb = ybl[k], ybl[k + 1]
                    n = yb - ya
                    psu = psS.tile([128, 64, 8], F32, tag="psS")
                    nc.tensor.matmul(
                        out=psu[:, :, 0:n],
                        lhsT=mat,
                        rhs=v[:, :, ya:yb],
                        start=True,
                        stop=True,
                    )
                    nc.scalar.activation(
                        out=dst[:, :, ya:yb], in_=psu[:, :, 0:n], func=AF.Copy
                    )

            for h in range(4):
                _geometry(nc, pools, gg, gbias, mbias, cb, G, h)
                for b2 in range(2):
                    bi = h * 2 + b2
                    _mac_band(nc, pools, bi, v, u, wd, G, ident, pwt, pwb, out_d)
    nc.compile()
    return nc


_NC_CACHE = {}


def _get_nc():
    if "nc" not in _NC_CACHE:
        _NC_CACHE["nc"] = build_nc()
    return _NC_CACHE["nc"]


def prep_core_inputs(inputs, core):
    x = np.ascontiguousarray(inputs["x"][core].reshape(C, L), dtype=np.float32)
    value_w = inputs["value_w"].astype(np.float32)
    gate_w = inputs["gate_w"].astype(np.float32)
    geom_w = inputs["geom_w"].astype(np.float32)
    pw_w = inputs["pw_w"].astype(np.float32)
    rhs0 = np.concatenate([value_w.T, gate_w.T, geom_w.T], axis=1)  # [64, 80]
    pw_t2 = np.ascontiguousarray(
        pw_w.T.reshape(2, 128, 64).transpose(1, 0, 2)
    )  # [128, 2, 64]; pw_t2[p, h, o] = pw_w[o, h*128+p]
    tri = np.eye(128, k=1, dtype=np.float32) + np.eye(128, k=-1, dtype=np.float32)
    wdt = np.eye(128, k=1, dtype=np.float32) - np.eye(128, k=-1, dtype=np.float32)
    shiftm = np.concatenate([tri, wdt], axis=1)
    return {
        "x": x,
        "shiftm": shiftm.astype(ml_dtypes.bfloat16),
        "rhs0": np.ascontiguousarray(rhs0).astype(ml_dtypes.bfloat16),
        "pw_t2": pw_t2.astype(ml_dtypes.bfloat16),
        "gate_b": inputs["gate_b"].astype(np.float32),
        "geom_b": inputs["geom_b"].astype(np.float32),
        "pw_b": inputs["pw_b"].astype(np.float32),
    }


def kernel(**inputs):
    from concourse.bass_utils import run_bass_kernel_spmd

    nc = _get_nc()
    inputs = {k: np.asarray(v) for k, v in inputs.items()}
    in_maps = [prep_core_inputs(inputs, i) for i in range(B)]
    res = run_bass_kernel_spmd(nc, in_maps, core_ids=list(range(B)))
    out = np.stack([r["out"].reshape(C, H, W) for r in res.results])
    return out.astype(np.float32)


if __name__ == "__main__":
    nc = build_nc()
    print("built ok")


# revision 28
# speedup vs baseline: 1.1059x; 1.1059x over previous
"""AZConv2d fused anisotropic conv kernel for Trainium2 (Bass/Tile), v3.

Math (per pixel l, rule r):
  gate = gate_w@x + gate_b; mu = softmax_r(gate)
  v = value_w@x;  geom = geom_w@x + geom_b -> theta, raw_base, raw_hyper (4 each)
  base = softplus(rb)+1e-4; hyper = softplus(rh)+0.1
  iu2 = 1/(base*e^h)^2; is2 = 1/(base*e^-h)^2
  kern(dy,dx) = exp(-(a*dx^2 + b*dy^2 + c2*dx*dy)),
     a = ct^2*iu2 + st^2*is2, b = st^2*iu2+ct^2*is2, c2 = 2*ct*st*(iu2-is2)
  w = mu*kern / (sum_{r,s} mu*kern + 1e-6)
  agg[r,c] = sum_s w[r,s] * v[c, l+delta_s];  out = pw_w @ agg + pw_b

With V0=v, V1=u=v(l-1)+v(l+1), V2=v(l-W)+v(l+W), S=u(l-W)+u(l+W),
T=wd(l-W)-wd(l+W), wd=v(l-1)-v(l+1):
  agg_r = G0*V0 + G1*V1 + G2*V2 + G3*S + G4*T,
  G0=em', G1=em'A, G2=em'B, G3=em'AB*cosh(c2), G4=-em'AB*sinh(c2).

v3: whole-image persistent fields in c-major [128 x-pixels, 64 chan, y]
layout (y-minor keeps every MAC operand unit-stride innermost -> DVE 2x
mode). x-shifts via partition-shifted sbuf->sbuf DMA (no x1/xd ops, no
u/w projections, no halo recompute). MAC [r, c, y] iteration, PE
transposes + pointwise as in the baseline.

Sharding: data-parallel over batch, 1 image per NeuronCore (8 cores).
"""

import math
import sys

for p in ("/opt/trn_rl_repo",):
    if p not in sys.path:
        sys.path.insert(0, p)

import ml_dtypes
import numpy as np

import concourse.bass as bass
import concourse.mybir as mybir
import concourse.tile as tile
from concourse import bacc
from concourse.bass import ds
from concourse.masks import make_identity

F32 = mybir.dt.float32
BF16 = mybir.dt.bfloat16
AF = mybir.ActivationFunctionType
ALU = mybir.AluOpType

B = 8
C = 64
H = 128
W = 128
L = H * W
R = 4
BAND = 8
NBANDS = H // BAND  # 16
EPS = 1e-4
MIN_HYP = 0.1
LN2 = math.log(2.0)


def _projection(nc, pools, x2, rhs0, v, gg, q):
    """Row projections for quad q (rows 32q..32q+31) -> v slots (c-major)
    and gg [128, 16, 128] f32 (gate 0:4 | theta 4:8 | rbase 8:12 | rhyp 12:16).
    """
    pps = pools["ps"]
    half = q // 2
    p0 = 64 * half
    rowbase = 0 if half == 0 else 63

    def xrow(y):
        return x2[p0 : p0 + 64, ds((y - rowbase) * W, W)]

    base = 32 * q
    for g in range(7):  # 7 groups of 5 rows (last group 2) per quad
        ra = base + g * 5
        rb = min(ra + 5, base + 32)
        n = rb - ra
        ps = pps.tile([128, 5, 80], F32, tag="psv")
        for j in range(n):
            nc.tensor.matmul(
                out=ps[:, j, :],
                lhsT=xrow(ra + j),
                rhs=rhs0[p0 : p0 + 64, :],
                start=True,
                stop=True,
            )
        # transposed copies: psum [y, c] -> sbuf [c, y]
        nc.scalar.activation(
            out=v[:, :, ra + 1 : rb + 1],
            in_=ps[:, 0:n, 0:64].rearrange("p y c -> p c y"),
            func=AF.Copy,
        )
        nc.scalar.activation(
            out=gg[:, :, ra:rb],
            in_=ps[:, 0:n, 64:80].rearrange("p y f -> p f y"),
            func=AF.Copy,
        )


def _shift_uw(nc, pools, shiftm, v, u, wd, q):
    """u/wd (PE pair sum/diff over the pixel index) for quad q's slot range."""
    psS = pools["psS"]
    lo = 32 * q if q > 0 else 0
    hi = 32 * q + 32 if q < 3 else 130
    ybl = list(range(lo, hi, 8)) + [hi]
    for which, mat, dst in ((0, shiftm[:, 0:128], u), (1, shiftm[:, 128:256], wd)):
        for k in range(len(ybl) - 1):
            ya, yb = ybl[k], ybl[k + 1]
            n = yb - ya
            psu = psS.tile([128, 64, 8], F32, tag="psS")
            nc.tensor.matmul(
                out=psu[:, :, 0:n],
                lhsT=mat,
                rhs=v[:, :, ya:yb],
                start=True,
                stop=True,
            )
            nc.scalar.activation(
                out=dst[:, :, ya:yb], in_=psu[:, :, 0:n], func=AF.Copy
            )


def _geometry(nc, pools, gg, gbias, mbias, cb, G, h):
    """Coefficients G[0..4] (each [128, 4, 128] bf16, y-minor) for quad h
    (rows 32h..32h+31) from gg [128, 16, 128] f32."""
    pg = pools["geo"]
    Y = 32
    yc = slice(32 * h, 32 * h + 32)

    def gt(tag, nf=4, dt=F32):
        return pg.tile([128, nf, Y], dt, tag=tag, name=tag)

    gate = gg[:, 0:4, yc]
    theta = gg[:, 4:8, yc]

    nc.vector.tensor_tensor(
        out=gate, in0=gate, in1=gbias[:, :, None].broadcast_to([128, 4, Y]),
        op=ALU.add,
    )
    nc.vector.tensor_tensor(
        out=gg[:, 4:16, yc],
        in0=gg[:, 4:16, yc],
        in1=mbias[:, :, None].broadcast_to([128, 12, Y]),
        op=ALU.add,
    )

    # trig; ct/st packed as cst = [ct | st]
    cst = gt("cst", 8)
    ct, st = cst[:, 0:4, :], cst[:, 4:8, :]
    sh = gt("sh")
    nc.scalar.activation(out=st, in_=theta, func=AF.Sin)
    nc.scalar.activation(out=sh, in_=theta, func=AF.Sin, scale=0.5)
    sh2 = gt("sh2")
    nc.vector.tensor_tensor(out=sh2, in0=sh, in1=sh, op=ALU.mult)
    nc.vector.tensor_scalar(
        out=ct, in0=sh2, scalar1=-2.0, scalar2=1.0, op0=ALU.mult, op1=ALU.add
    )

    # softmax numerator: em = exp(gate - max_r gate)
    mx = pg.tile([128, 1, Y], F32, tag="mx")
    m01 = pg.tile([128, 1, Y], F32, tag="m01")
    m23 = pg.tile([128, 1, Y], F32, tag="m23")
    nc.vector.tensor_tensor(
        out=m01, in0=gate[:, 0:1, :], in1=gate[:, 1:2, :], op=ALU.max
    )
    nc.vector.tensor_tensor(
        out=m23, in0=gate[:, 2:3, :], in1=gate[:, 3:4, :], op=ALU.max
    )
    nc.vector.tensor_tensor(out=mx, in0=m01, in1=m23, op=ALU.max)
    gsh = gt("gsh")
    nc.vector.tensor_tensor(
        out=gsh, in0=gate, in1=mx.broadcast_to([128, 4, Y]), op=ALU.subtract
    )
    em = gt("em")
    nc.scalar.activation(out=em, in_=gsh, func=AF.Exp)

    # softplus(base|hyper) = ln(1 + exp(.)), then ln(base+eps)
    e8 = gt("e8", 8)
    nc.scalar.activation(out=e8, in_=gg[:, 8:16, yc], func=AF.Exp)
    sp8 = gt("sp8", 8)
    nc.scalar.activation(out=sp8, in_=e8, func=AF.Ln, bias=1.0)
    spb, sph = sp8[:, 0:4, :], sp8[:, 4:8, :]
    lb = gt("lb")
    nc.scalar.activation(out=lb, in_=spb, func=AF.Ln, bias=cb["eps"])

    tpl, tmi = gt("tpl"), gt("tmi")
    nc.vector.tensor_tensor(out=tpl, in0=sph, in1=lb, op=ALU.add)
    nc.vector.tensor_tensor(out=tmi, in0=sph, in1=lb, op=ALU.subtract)
    ii = gt("ii", 8)
    iu2, is2 = ii[:, 0:4, :], ii[:, 4:8, :]
    nc.scalar.activation(out=iu2, in_=tpl, func=AF.Exp, scale=-2.0, bias=cb["mh2n"])
    nc.scalar.activation(out=is2, in_=tmi, func=AF.Exp, scale=2.0, bias=cb["mh2p"])

    sq = gt("sq", 8)
    nc.vector.tensor_tensor(out=sq, in0=cst, in1=cst, op=ALU.mult)
    t12 = gt("t12", 8)
    nc.vector.tensor_tensor(out=t12, in0=sq, in1=ii, op=ALU.mult)
    av = gt("av")
    nc.vector.tensor_tensor(out=av, in0=t12[:, 0:4, :], in1=t12[:, 4:8, :], op=ALU.add)
    ssum, bv = gt("ssum"), gt("bv")
    nc.vector.tensor_tensor(out=ssum, in0=iu2, in1=is2, op=ALU.add)
    nc.vector.tensor_tensor(out=bv, in0=ssum, in1=av, op=ALU.subtract)

    cs, dio, c2h = gt("cs"), gt("dio"), gt("c2h")
    nc.vector.tensor_tensor(out=cs, in0=ct, in1=st, op=ALU.mult)
    nc.vector.tensor_tensor(out=dio, in0=iu2, in1=is2, op=ALU.subtract)
    nc.vector.tensor_tensor(out=c2h, in0=cs, in1=dio, op=ALU.mult)

    Aa, Bb = gt("Aa"), gt("Bb")
    nc.scalar.activation(out=Aa, in_=av, func=AF.Exp, scale=-1.0)
    nc.scalar.activation(out=Bb, in_=bv, func=AF.Exp, scale=-1.0)
    # Corner kernels, overflow-safe: a+b = ssum >= |c2|
    sc2, sc2m = gt("sc2"), gt("sc2m")
    nc.vector.scalar_tensor_tensor(
        out=sc2, in0=c2h, scalar=2.0, in1=ssum, op0=ALU.mult, op1=ALU.add
    )
    nc.vector.scalar_tensor_tensor(
        out=sc2m, in0=c2h, scalar=-2.0, in1=ssum, op0=ALU.mult, op1=ALU.add
    )
    ph, qh = gt("ph"), gt("qh")
    nc.scalar.activation(out=ph, in_=sc2, func=AF.Exp, scale=-1.0, bias=cb["mln2"])
    nc.scalar.activation(out=qh, in_=sc2m, func=AF.Exp, scale=-1.0, bias=cb["mln2"])
    ppqh, pmqh = gt("ppqh"), gt("pmqh")
    nc.vector.tensor_tensor(out=ppqh, in0=ph, in1=qh, op=ALU.add)
    nc.vector.tensor_tensor(out=pmqh, in0=ph, in1=qh, op=ALU.subtract)

    # Sk = 1 + 2(A+B) + 4*(P+Q)/2
    apb, w1, sk = gt("apb"), gt("w1"), gt("sk")
    nc.vector.tensor_tensor(out=apb, in0=Aa, in1=Bb, op=ALU.add)
    nc.vector.scalar_tensor_tensor(
        out=w1, in0=ppqh, scalar=2.0, in1=apb, op0=ALU.mult, op1=ALU.add
    )
    nc.vector.tensor_scalar(
        out=sk, in0=w1, scalar1=2.0, scalar2=1.0, op0=ALU.mult, op1=ALU.add
    )

    ws = gt("ws")
    nc.vector.tensor_tensor(out=ws, in0=em, in1=sk, op=ALU.mult)
    d01 = pg.tile([128, 1, Y], F32, tag="d01", name="d01")
    d23 = pg.tile([128, 1, Y], F32, tag="d23", name="d23")
    Dp = pg.tile([128, 1, Y], F32, tag="Dp", name="Dp")
    nc.vector.tensor_tensor(out=d01, in0=ws[:, 0:1, :], in1=ws[:, 1:2, :], op=ALU.add)
    nc.vector.tensor_tensor(out=d23, in0=ws[:, 2:3, :], in1=ws[:, 3:4, :], op=ALU.add)
    nc.vector.tensor_tensor(out=Dp, in0=d01, in1=d23, op=ALU.add)
    s01 = pg.tile([128, 1, Y], F32, tag="s01", name="s01")
    s23 = pg.tile([128, 1, Y], F32, tag="s23", name="s23")
    Smu = pg.tile([128, 1, Y], F32, tag="Smu", name="Smu")
    nc.vector.tensor_tensor(out=s01, in0=em[:, 0:1, :], in1=em[:, 1:2, :], op=ALU.add)
    nc.vector.tensor_tensor(out=s23, in0=em[:, 2:3, :], in1=em[:, 3:4, :], op=ALU.add)
    nc.vector.tensor_tensor(out=Smu, in0=s01, in1=s23, op=ALU.add)
    D2 = pg.tile([128, 1, Y], F32, tag="D2")
    nc.vector.scalar_tensor_tensor(
        out=D2, in0=Smu, scalar=1e-6, in1=Dp, op0=ALU.mult, op1=ALU.add
    )
    invD = pg.tile([128, 1, Y], F32, tag="invD")
    nc.vector.reciprocal(invD, D2)

    invDb = invD.broadcast_to([128, 4, Y])
    nc.vector.tensor_tensor(out=G[0][:, :, yc], in0=em, in1=invDb, op=ALU.mult)
    em2 = G[0][:, :, yc]
    nc.vector.tensor_tensor(out=G[1][:, :, yc], in0=em2, in1=Aa, op=ALU.mult)
    nc.vector.tensor_tensor(out=G[2][:, :, yc], in0=em2, in1=Bb, op=ALU.mult)
    nc.vector.tensor_tensor(out=G[3][:, :, yc], in0=em2, in1=ppqh, op=ALU.mult)
    nc.vector.tensor_tensor(out=G[4][:, :, yc], in0=em2, in1=pmqh, op=ALU.mult)


def _mac_band(nc, pools, bi, v, u, wd, G, ident, pwt, pwb, out_d):
    """MAC + PE transpose + pointwise for band pair bi (rows y0..y0+15)."""
    pm, ppsT, ppso, pout = pools["mac"], pools["psT"], pools["pso"], pools["out"]
    NY = 2 * BAND
    y0 = bi * NY
    sh = [128, R, 64, NY]  # [x, r, c, y]

    def bf(t, off):  # field [128, 64, 130] -> [r, c, y] view (bcast r)
        return t[:, None, :, y0 + off : y0 + off + NY].broadcast_to(sh)

    def bg(t):  # coeff [128, 4, 128] -> [r, c, y] view (bcast c)
        return t[:, :, None, y0 : y0 + NY].broadcast_to(sh)

    # v2/vs/vt for this band pair (c-major, y-minor views of whole fields)
    v2 = pm.tile([128, 64, NY], BF16, tag="v2")
    vs = pm.tile([128, 64, NY], BF16, tag="vs")
    vt = pm.tile([128, 64, NY], BF16, tag="vt")
    nc.vector.tensor_tensor(
        out=v2, in0=v[:, :, y0 : y0 + NY], in1=v[:, :, y0 + 2 : y0 + NY + 2],
        op=ALU.add,
    )
    nc.vector.tensor_tensor(
        out=vs, in0=u[:, :, y0 : y0 + NY], in1=u[:, :, y0 + 2 : y0 + NY + 2],
        op=ALU.add,
    )
    nc.vector.tensor_tensor(
        out=vt, in0=wd[:, :, y0 : y0 + NY], in1=wd[:, :, y0 + 2 : y0 + NY + 2],
        op=ALU.subtract,
    )

    def bb(t):  # band field [128, 64, 16] -> [r, c, y]
        return t[:, None, :, :].broadcast_to(sh)

    agg = pm.tile(sh, BF16, tag="agg")
    tA = pm.tile(sh, BF16, tag="tA")
    tB = pm.tile(sh, BF16, tag="tB")
    nc.vector.tensor_tensor(out=tA, in0=bf(v, 1), in1=bg(G[0]), op=ALU.mult)
    nc.vector.tensor_tensor(out=tB, in0=bf(u, 1), in1=bg(G[1]), op=ALU.mult)
    nc.vector.tensor_tensor(out=agg, in0=tA, in1=tB, op=ALU.add)
    nc.vector.tensor_tensor(out=tA, in0=bb(v2), in1=bg(G[2]), op=ALU.mult)
    nc.vector.tensor_tensor(out=tB, in0=bb(vs), in1=bg(G[3]), op=ALU.mult)
    nc.vector.tensor_tensor(out=agg, in0=agg, in1=tA, op=ALU.add)
    nc.vector.tensor_tensor(out=tA, in0=bb(vt), in1=bg(G[4]), op=ALU.mult)
    nc.vector.tensor_tensor(out=agg, in0=agg, in1=tB, op=ALU.add)
    nc.vector.tensor_tensor(out=agg, in0=agg, in1=tA, op=ALU.add)

    # transpose [pixel, (r c)] -> [(r c), pixel] via PE, half (128 rc) at a time
    aggT = pm.tile([128, 2, NY, 128], BF16, tag="aggT")
    for hg in range(2):
        for grp in range(NY // 4):
            psT = ppsT.tile([128, 4, 128], BF16, tag="psT")
            for yy in range(4):
                yr = grp * 4 + yy
                nc.tensor.transpose(
                    out=psT[:, yy, :],
                    in_=agg[:, 2 * hg : 2 * hg + 2, :, yr].rearrange(
                        "p a b -> p (a b)"
                    ),
                    identity=ident,
                )
            nc.scalar.activation(
                out=aggT[:, hg, grp * 4 : grp * 4 + 4, :], in_=psT, func=AF.Copy
            )

    # out = pw @ agg + pw_b (c-major), 512 pixels per psum chunk
    for ch in range(NY * W // 512):
        ps_o = ppso.tile([64, 512], F32, tag="pso")
        for hg in range(2):
            nc.tensor.matmul(
                out=ps_o,
                lhsT=pwt[:, hg, :],
                rhs=aggT[:, hg, ch * 4 : ch * 4 + 4, :].rearrange("p a b -> p (a b)"),
                start=(hg == 0),
                stop=(hg == 1),
            )
        osb = pout.tile([64, 512], F32, tag="osb")
        nc.scalar.activation(out=osb, in_=ps_o, func=AF.Identity, bias=pwb, scale=1.0)
        # alternate issuing engine so output DMAs use two hardware queues
        eng = nc.sync if (bi + ch) % 2 == 0 else nc.scalar
        eng.dma_start(out=out_d[:, ds(y0 * W + ch * 512, 512)], in_=osb)


def build_nc():
    nc = bacc.Bacc("TRN2")
    x_d = nc.dram_tensor("x", [C, L], F32, kind="ExternalInput").ap()
    sh_d = nc.dram_tensor("shiftm", [128, 256], BF16, kind="ExternalInput").ap()
    rhs0_d = nc.dram_tensor("rhs0", [C, 80], BF16, kind="ExternalInput").ap()
    pwt_d = nc.dram_tensor("pw_t2", [128, 2, 64], BF16, kind="ExternalInput").ap()
    gb_d = nc.dram_tensor("gate_b", [R], F32, kind="ExternalInput").ap()
    mb_d = nc.dram_tensor("geom_b", [12], F32, kind="ExternalInput").ap()
    pb_d = nc.dram_tensor("pw_b", [C], F32, kind="ExternalInput").ap()
    out_d = nc.dram_tensor("out", [C, L], F32, kind="ExternalOutput").ap()

    with tile.TileContext(nc) as tc:
        import contextlib

        with contextlib.ExitStack() as ctx:
            persist = ctx.enter_context(tc.tile_pool(name="persist", bufs=1))
            pools = {
                "geo": ctx.enter_context(tc.tile_pool(name="geo", bufs=2)),
                "mac": ctx.enter_context(tc.tile_pool(name="mac", bufs=2)),
                "out": ctx.enter_context(tc.tile_pool(name="out", bufs=2)),
                "psS": ctx.enter_context(
                    tc.tile_pool(name="psS", bufs=2, space="PSUM")
                ),
                "ps": ctx.enter_context(tc.tile_pool(name="ps", bufs=2, space="PSUM")),
                "psT": ctx.enter_context(
                    tc.tile_pool(name="psT", bufs=2, space="PSUM")
                ),
                "pso": ctx.enter_context(
                    tc.tile_pool(name="pso", bufs=2, space="PSUM")
                ),
            }

            x2 = persist.tile([128, 65 * W], BF16)
            rhs0 = persist.tile([128, 80], BF16)
            pwt = persist.tile([128, 2, 64], BF16)
            gbias = persist.tile([128, R], F32)
            mbias = persist.tile([128, 12], F32)
            pwb = persist.tile([64, 1], F32)
            ident = persist.tile([128, 128], BF16)
            make_identity(nc, ident)
            shiftm = persist.tile([128, 256], BF16)
            nc.sync.dma_start(out=shiftm, in_=sh_d)
            v = persist.tile([128, 64, 130], BF16)
            u = persist.tile([128, 64, 130], BF16)
            wd = persist.tile([128, 64, 130], BF16)
            gg = persist.tile([128, 16, 128], F32)
            G = [persist.tile([128, 4, 128], BF16, name=f"G{i}") for i in range(5)]

            cb = {}
            for cname, cval in [
                ("eps", EPS),
                ("mh2n", -2.0 * MIN_HYP),
                ("mh2p", 2.0 * MIN_HYP),
                ("mln2", -LN2),
            ]:
                t = persist.tile([128, 1], F32, tag="cb_" + cname, name="cb_" + cname)
                nc.gpsimd.memset(t, cval)
                cb[cname] = t

            CH = 65 * W // 8
            for c8 in range(8):
                nc.gpsimd.dma_start(
                    out=x2[0:64, ds(c8 * CH, CH)], in_=x_d[:, ds(c8 * CH, CH)]
                )
                nc.gpsimd.dma_start(
                    out=x2[64:128, ds(c8 * CH, CH)],
                    in_=x_d[:, ds(63 * W + c8 * CH, CH)],
                )
            nc.sync.dma_start(out=rhs0[0:64, :], in_=rhs0_d)
            nc.sync.dma_start(out=rhs0[64:128, :], in_=rhs0_d)
            nc.sync.dma_start(out=pwt, in_=pwt_d)
            nc.gpsimd.dma_start(
                out=gbias,
                in_=bass.AP(tensor=gb_d.tensor, offset=0, ap=[[0, 128], [1, R]]),
            )
            nc.gpsimd.dma_start(
                out=mbias,
                in_=bass.AP(tensor=mb_d.tensor, offset=0, ap=[[0, 128], [1, 12]]),
            )
            nc.sync.dma_start(out=pwb, in_=pb_d.rearrange("(c o) -> c o", o=1))

            # y halos (disjoint from projection writes, so order-free)
            nc.gpsimd.memset(v[:, :, 0:1], 0.0)
            nc.gpsimd.memset(v[:, :, 129:130], 0.0)

            # software-pipelined per-quad emission keeps every engine's
            # in-order stream free of long cross-stage stalls
            _projection(nc, pools, x2, rhs0, v, gg, 0)
            _shift_uw(nc, pools, shiftm, v, u, wd, 0)
            _geometry(nc, pools, gg, gbias, mbias, cb, G, 0)
            _projection(nc, pools, x2, rhs0, v, gg, 1)
            _shift_uw(nc, pools, shiftm, v, u, wd, 1)
            for h in range(4):
                for b2 in range(2):
                    bi = h * 2 + b2
                    _mac_band(nc, pools, bi, v, u, wd, G, ident, pwt, pwb, out_d)
                if h + 2 <= 3:
                    _projection(nc, pools, x2, rhs0, v, gg, h + 2)
                    _shift_uw(nc, pools, shiftm, v, u, wd, h + 2)
                if h + 1 <= 3:
                    _geometry(nc, pools, gg, gbias, mbias, cb, G, h + 1)
    nc.compile()
    return nc


_NC_CACHE = {}


def _get_nc():
    if "nc" not in _NC_CACHE:
        _NC_CACHE["nc"] = build_nc()
    return _NC_CACHE["nc"]


def prep_core_inputs(inputs, core):
    x = np.ascontiguousarray(inputs["x"][core].reshape(C, L), dtype=np.float32)
    value_w = inputs["value_w"].astype(np.float32)
    gate_w = inputs["gate_w"].astype(np.float32)
    geom_w = inputs["geom_w"].astype(np.float32)
    pw_w = inputs["pw_w"].astype(np.float32)
    rhs0 = np.concatenate([value_w.T, gate_w.T, geom_w.T], axis=1)  # [64, 80]
    pw_t2 = np.ascontiguousarray(
        pw_w.T.reshape(2, 128, 64).transpose(1, 0, 2)
    )  # [128, 2, 64]; pw_t2[p, h, o] = pw_w[o, h*128+p]
    tri = np.eye(128, k=1, dtype=np.float32) + np.eye(128, k=-1, dtype=np.float32)
    wdt = np.eye(128, k=1, dtype=np.float32) - np.eye(128, k=-1, dtype=np.float32)
    shiftm = np.concatenate([tri, wdt], axis=1)
    return {
        "x": x,
        "shiftm": shiftm.astype(ml_dtypes.bfloat16),
        "rhs0": np.ascontiguousarray(rhs0).astype(ml_dtypes.bfloat16),
        "pw_t2": pw_t2.astype(ml_dtypes.bfloat16),
        "gate_b": inputs["gate_b"].astype(np.float32),
        "geom_b": inputs["geom_b"].astype(np.float32),
        "pw_b": inputs["pw_b"].astype(np.float32),
    }


def kernel(**inputs):
    from concourse.bass_utils import run_bass_kernel_spmd

    nc = _get_nc()
    inputs = {k: np.asarray(v) for k, v in inputs.items()}
    in_maps = [prep_core_inputs(inputs, i) for i in range(B)]
    res = run_bass_kernel_spmd(nc, in_maps, core_ids=list(range(B)))
    out = np.stack([r["out"].reshape(C, H, W) for r in res.results])
    return out.astype(np.float32)


if __name__ == "__main__":
    nc = build_nc()
    print("built ok")


# revision 37
# speedup vs baseline: 1.1239x; 1.0163x over previous
"""AZConv2d fused anisotropic conv kernel for Trainium2 (Bass/Tile), v3.

Math (per pixel l, rule r):
  gate = gate_w@x + gate_b; mu = softmax_r(gate)
  v = value_w@x;  geom = geom_w@x + geom_b -> theta, raw_base, raw_hyper (4 each)
  base = softplus(rb)+1e-4; hyper = softplus(rh)+0.1
  iu2 = 1/(base*e^h)^2; is2 = 1/(base*e^-h)^2
  kern(dy,dx) = exp(-(a*dx^2 + b*dy^2 + c2*dx*dy)),
     a = ct^2*iu2 + st^2*is2, b = st^2*iu2+ct^2*is2, c2 = 2*ct*st*(iu2-is2)
  w = mu*kern / (sum_{r,s} mu*kern + 1e-6)
  agg[r,c] = sum_s w[r,s] * v[c, l+delta_s];  out = pw_w @ agg + pw_b

With V0=v, V1=u=v(l-1)+v(l+1), V2=v(l-W)+v(l+W), S=u(l-W)+u(l+W),
T=wd(l-W)-wd(l+W), wd=v(l-1)-v(l+1):
  agg_r = G0*V0 + G1*V1 + G2*V2 + G3*S + G4*T,
  G0=em', G1=em'A, G2=em'B, G3=em'AB*cosh(c2), G4=-em'AB*sinh(c2).

v3: whole-image persistent fields in c-major [128 x-pixels, 64 chan, y]
layout (y-minor keeps every MAC operand unit-stride innermost -> DVE 2x
mode). x-shifts via partition-shifted sbuf->sbuf DMA (no x1/xd ops, no
u/w projections, no halo recompute). MAC [r, c, y] iteration, PE
transposes + pointwise as in the baseline.

Sharding: data-parallel over batch, 1 image per NeuronCore (8 cores).
"""

import math
import sys

for p in ("/opt/trn_rl_repo",):
    if p not in sys.path:
        sys.path.insert(0, p)

import ml_dtypes
import numpy as np

import concourse.bass as bass
import concourse.mybir as mybir
import concourse.tile as tile
from concourse import bacc
from concourse.bass import ds
from concourse.masks import make_identity

F32 = mybir.dt.float32
BF16 = mybir.dt.bfloat16
AF = mybir.ActivationFunctionType
ALU = mybir.AluOpType

B = 8
C = 64
H = 128
W = 128
L = H * W
R = 4
BAND = 8
NBANDS = H // BAND  # 16
EPS = 1e-4
MIN_HYP = 0.1
LN2 = math.log(2.0)


def _projection(nc, pools, x2, rhs0, v, gg, q):
    """Row projections for quad q (rows 32q..32q+31) -> v slots (c-major)
    and gg [128, 16, 128] f32 (gate 0:4 | theta 4:8 | rbase 8:12 | rhyp 12:16).
    """
    pps = pools["ps"]
    half = q // 2
    p0 = 64 * half
    rowbase = 0 if half == 0 else 63

    def xrow(y):
        return x2[p0 : p0 + 64, ds((y - rowbase) * W, W)]

    base = 32 * q
    for g in range(7):  # 7 groups of 5 rows (last group 2) per quad
        ra = base + g * 5
        rb = min(ra + 5, base + 32)
        n = rb - ra
        ps = pps.tile([128, 5, 80], F32, tag="psv")
        for j in range(n):
            nc.tensor.matmul(
                out=ps[:, j, :],
                lhsT=xrow(ra + j),
                rhs=rhs0[p0 : p0 + 64, :],
                start=True,
                stop=True,
            )
        # transposed copies: psum [y, c] -> sbuf [c, y]
        nc.scalar.activation(
            out=v[:, :, ra + 1 : rb + 1],
            in_=ps[:, 0:n, 0:64].rearrange("p y c -> p c y"),
            func=AF.Copy,
        )
        nc.scalar.activation(
            out=gg[:, :, ra:rb],
            in_=ps[:, 0:n, 64:80].rearrange("p y f -> p f y"),
            func=AF.Copy,
        )


def _shift_uw(nc, pools, shiftm, v, u, wd, q):
    """u/wd (PE pair sum/diff over the pixel index) for quad q's slot range."""
    psS = pools["psS"]
    lo = 32 * q if q > 0 else 0
    hi = 32 * q + 32 if q < 3 else 130
    ybl = list(range(lo, hi, 8)) + [hi]
    for which, mat, dst in ((0, shiftm[:, 0:128], u), (1, shiftm[:, 128:256], wd)):
        for k in range(len(ybl) - 1):
            ya, yb = ybl[k], ybl[k + 1]
            n = yb - ya
            psu = psS.tile([128, 64, 8], F32, tag="psS")
            nc.tensor.matmul(
                out=psu[:, :, 0:n],
                lhsT=mat,
                rhs=v[:, :, ya:yb],
                start=True,
                stop=True,
            )
            nc.scalar.activation(
                out=dst[:, :, ya:yb], in_=psu[:, :, 0:n], func=AF.Copy
            )


def _geometry(nc, pools, gg, gbias, mbias, cb, G, h):
    """Coefficients G[0..4] (each [128, 4, 128] bf16, y-minor) for quad h
    (rows 32h..32h+31) from gg [128, 16, 128] f32."""
    pg = pools["geo"]
    Y = 32
    yc = slice(32 * h, 32 * h + 32)

    def gt(tag, nf=4, dt=F32):
        return pg.tile([128, nf, Y], dt, tag=tag, name=tag)

    gate = gg[:, 0:4, yc]
    theta = gg[:, 4:8, yc]

    nc.vector.tensor_tensor(
        out=gate, in0=gate, in1=gbias[:, :, None].broadcast_to([128, 4, Y]),
        op=ALU.add,
    )
    nc.vector.tensor_tensor(
        out=gg[:, 4:16, yc],
        in0=gg[:, 4:16, yc],
        in1=mbias[:, :, None].broadcast_to([128, 12, Y]),
        op=ALU.add,
    )

    # trig; ct/st packed as cst = [ct | st]
    cst = gt("cst", 8)
    ct, st = cst[:, 0:4, :], cst[:, 4:8, :]
    sh = gt("sh")
    nc.scalar.activation(out=st, in_=theta, func=AF.Sin)
    nc.scalar.activation(out=sh, in_=theta, func=AF.Sin, scale=0.5)
    sh2 = gt("sh2")
    nc.vector.tensor_tensor(out=sh2, in0=sh, in1=sh, op=ALU.mult)
    nc.vector.tensor_scalar(
        out=ct, in0=sh2, scalar1=-2.0, scalar2=1.0, op0=ALU.mult, op1=ALU.add
    )

    # softmax numerator without the max shift: the normalization cancels any
    # shift exactly, and |gate| stays O(1) here so exp cannot overflow
    em = gt("em")
    nc.scalar.activation(out=em, in_=gate, func=AF.Exp)

    # softplus(base|hyper) = ln(1 + exp(.)), then ln(base+eps)
    e8 = gt("e8", 8)
    nc.scalar.activation(out=e8, in_=gg[:, 8:16, yc], func=AF.Exp)
    sp8 = gt("sp8", 8)
    nc.scalar.activation(out=sp8, in_=e8, func=AF.Ln, bias=1.0)
    spb, sph = sp8[:, 0:4, :], sp8[:, 4:8, :]
    lb = gt("lb")
    nc.scalar.activation(out=lb, in_=spb, func=AF.Ln, bias=cb["eps"])

    tpl, tmi = gt("tpl"), gt("tmi")
    nc.vector.tensor_tensor(out=tpl, in0=sph, in1=lb, op=ALU.add)
    nc.vector.tensor_tensor(out=tmi, in0=sph, in1=lb, op=ALU.subtract)
    ii = gt("ii", 8)
    iu2, is2 = ii[:, 0:4, :], ii[:, 4:8, :]
    nc.scalar.activation(out=iu2, in_=tpl, func=AF.Exp, scale=-2.0, bias=cb["mh2n"])
    nc.scalar.activation(out=is2, in_=tmi, func=AF.Exp, scale=2.0, bias=cb["mh2p"])

    sq = gt("sq", 8)
    nc.vector.tensor_tensor(out=sq, in0=cst, in1=cst, op=ALU.mult)
    t12 = gt("t12", 8)
    nc.vector.tensor_tensor(out=t12, in0=sq, in1=ii, op=ALU.mult)
    av = gt("av")
    nc.vector.tensor_tensor(out=av, in0=t12[:, 0:4, :], in1=t12[:, 4:8, :], op=ALU.add)
    ssum, bv = gt("ssum"), gt("bv")
    nc.vector.tensor_tensor(out=ssum, in0=iu2, in1=is2, op=ALU.add)
    nc.vector.tensor_tensor(out=bv, in0=ssum, in1=av, op=ALU.subtract)

    cs, dio, c2h = gt("cs"), gt("dio"), gt("c2h")
    nc.vector.tensor_tensor(out=cs, in0=ct, in1=st, op=ALU.mult)
    nc.vector.tensor_tensor(out=dio, in0=iu2, in1=is2, op=ALU.subtract)
    nc.vector.tensor_tensor(out=c2h, in0=cs, in1=dio, op=ALU.mult)

    Aa, Bb = gt("Aa"), gt("Bb")
    nc.scalar.activation(out=Aa, in_=av, func=AF.Exp, scale=-1.0)
    nc.scalar.activation(out=Bb, in_=bv, func=AF.Exp, scale=-1.0)
    # Corner kernels, overflow-safe: a+b = ssum >= |c2|
    sc2, sc2m = gt("sc2"), gt("sc2m")
    nc.vector.scalar_tensor_tensor(
        out=sc2, in0=c2h, scalar=2.0, in1=ssum, op0=ALU.mult, op1=ALU.add
    )
    nc.vector.scalar_tensor_tensor(
        out=sc2m, in0=c2h, scalar=-2.0, in1=ssum, op0=ALU.mult, op1=ALU.add
    )
    ph, qh = gt("ph"), gt("qh")
    nc.scalar.activation(out=ph, in_=sc2, func=AF.Exp, scale=-1.0, bias=cb["mln2"])
    nc.scalar.activation(out=qh, in_=sc2m, func=AF.Exp, scale=-1.0, bias=cb["mln2"])
    ppqh, pmqh = gt("ppqh"), gt("pmqh")
    nc.vector.tensor_tensor(out=ppqh, in0=ph, in1=qh, op=ALU.add)
    nc.vector.tensor_tensor(out=pmqh, in0=ph, in1=qh, op=ALU.subtract)

    # Sk = 1 + 2(A+B) + 4*(P+Q)/2
    apb, w1, sk = gt("apb"), gt("w1"), gt("sk")
    nc.vector.tensor_tensor(out=apb, in0=Aa, in1=Bb, op=ALU.add)
    nc.vector.scalar_tensor_tensor(
        out=w1, in0=ppqh, scalar=2.0, in1=apb, op0=ALU.mult, op1=ALU.add
    )
    nc.vector.tensor_scalar(
        out=sk, in0=w1, scalar1=2.0, scalar2=1.0, op0=ALU.mult, op1=ALU.add
    )

    ws = gt("ws")
    nc.vector.tensor_tensor(out=ws, in0=em, in1=sk, op=ALU.mult)
    d01 = pg.tile([128, 1, Y], F32, tag="d01", name="d01")
    d23 = pg.tile([128, 1, Y], F32, tag="d23", name="d23")
    Dp = pg.tile([128, 1, Y], F32, tag="Dp", name="Dp")
    nc.vector.tensor_tensor(out=d01, in0=ws[:, 0:1, :], in1=ws[:, 1:2, :], op=ALU.add)
    nc.vector.tensor_tensor(out=d23, in0=ws[:, 2:3, :], in1=ws[:, 3:4, :], op=ALU.add)
    nc.vector.tensor_tensor(out=Dp, in0=d01, in1=d23, op=ALU.add)
    s01 = pg.tile([128, 1, Y], F32, tag="s01", name="s01")
    s23 = pg.tile([128, 1, Y], F32, tag="s23", name="s23")
    Smu = pg.tile([128, 1, Y], F32, tag="Smu", name="Smu")
    nc.vector.tensor_tensor(out=s01, in0=em[:, 0:1, :], in1=em[:, 1:2, :], op=ALU.add)
    nc.vector.tensor_tensor(out=s23, in0=em[:, 2:3, :], in1=em[:, 3:4, :], op=ALU.add)
    nc.vector.tensor_tensor(out=Smu, in0=s01, in1=s23, op=ALU.add)
    D2 = pg.tile([128, 1, Y], F32, tag="D2")
    nc.vector.scalar_tensor_tensor(
        out=D2, in0=Smu, scalar=1e-6, in1=Dp, op0=ALU.mult, op1=ALU.add
    )
    invD = pg.tile([128, 1, Y], F32, tag="invD")
    nc.vector.reciprocal(invD, D2)

    invDb = invD.broadcast_to([128, 4, Y])
    nc.vector.tensor_tensor(out=G[0][:, :, yc], in0=em, in1=invDb, op=ALU.mult)
    em2 = G[0][:, :, yc]
    nc.vector.tensor_tensor(out=G[1][:, :, yc], in0=em2, in1=Aa, op=ALU.mult)
    nc.vector.tensor_tensor(out=G[2][:, :, yc], in0=em2, in1=Bb, op=ALU.mult)
    nc.vector.tensor_tensor(out=G[3][:, :, yc], in0=em2, in1=ppqh, op=ALU.mult)
    nc.vector.tensor_tensor(out=G[4][:, :, yc], in0=em2, in1=pmqh, op=ALU.mult)


def _mac_band(nc, pools, y0, NY, v, u, wd, G, ident, pwt, pwb, out_d):
    """MAC + PE transpose + pointwise for rows y0..y0+NY-1."""
    pm, ppsT, ppso, pout = pools["mac"], pools["psT"], pools["pso"], pools["out"]
    sh = [128, R, 64, NY]  # [x, r, c, y]

    def bf(t, off):  # field [128, 64, 130] -> [r, c, y] view (bcast r)
        return t[:, None, :, y0 + off : y0 + off + NY].broadcast_to(sh)

    def bg(t):  # coeff [128, 4, 128] -> [r, c, y] view (bcast c)
        return t[:, :, None, y0 : y0 + NY].broadcast_to(sh)

    # v2/vs/vt for this band pair (c-major, y-minor views of whole fields)
    v2 = pm.tile([128, 64, NY], BF16, tag="v2")
    vs = pm.tile([128, 64, NY], BF16, tag="vs")
    vt = pm.tile([128, 64, NY], BF16, tag="vt")
    nc.vector.tensor_tensor(
        out=v2, in0=v[:, :, y0 : y0 + NY], in1=v[:, :, y0 + 2 : y0 + NY + 2],
        op=ALU.add,
    )
    nc.vector.tensor_tensor(
        out=vs, in0=u[:, :, y0 : y0 + NY], in1=u[:, :, y0 + 2 : y0 + NY + 2],
        op=ALU.add,
    )
    nc.vector.tensor_tensor(
        out=vt, in0=wd[:, :, y0 : y0 + NY], in1=wd[:, :, y0 + 2 : y0 + NY + 2],
        op=ALU.subtract,
    )

    def bb(t):  # band field [128, 64, 16] -> [r, c, y]
        return t[:, None, :, :].broadcast_to(sh)

    agg = pm.tile(sh, BF16, tag="agg")
    tA = pm.tile(sh, BF16, tag="tA")
    tB = pm.tile(sh, BF16, tag="tB")
    nc.vector.tensor_tensor(out=tA, in0=bf(v, 1), in1=bg(G[0]), op=ALU.mult)
    nc.vector.tensor_tensor(out=tB, in0=bf(u, 1), in1=bg(G[1]), op=ALU.mult)
    nc.vector.tensor_tensor(out=agg, in0=tA, in1=tB, op=ALU.add)
    nc.vector.tensor_tensor(out=tA, in0=bb(v2), in1=bg(G[2]), op=ALU.mult)
    nc.vector.tensor_tensor(out=tB, in0=bb(vs), in1=bg(G[3]), op=ALU.mult)
    nc.vector.tensor_tensor(out=agg, in0=agg, in1=tA, op=ALU.add)
    nc.vector.tensor_tensor(out=tA, in0=bb(vt), in1=bg(G[4]), op=ALU.mult)
    nc.vector.tensor_tensor(out=agg, in0=agg, in1=tB, op=ALU.add)
    nc.vector.tensor_tensor(out=agg, in0=agg, in1=tA, op=ALU.add)

    # transpose [pixel, (r c)] -> [(r c), pixel] via PE, half (128 rc) at a time
    aggT = pm.tile([128, 2, NY, 128], BF16, tag="aggT")
    for hg in range(2):
        for grp in range(NY // 4):
            psT = ppsT.tile([128, 4, 128], BF16, tag="psT")
            for yy in range(4):
                yr = grp * 4 + yy
                nc.tensor.transpose(
                    out=psT[:, yy, :],
                    in_=agg[:, 2 * hg : 2 * hg + 2, :, yr].rearrange(
                        "p a b -> p (a b)"
                    ),
                    identity=ident,
                )
            nc.scalar.activation(
                out=aggT[:, hg, grp * 4 : grp * 4 + 4, :], in_=psT, func=AF.Copy
            )

    # out = pw @ agg + pw_b (c-major), 512 pixels per psum chunk
    for ch in range(NY * W // 512):
        ps_o = ppso.tile([64, 512], F32, tag="pso")
        for hg in range(2):
            nc.tensor.matmul(
                out=ps_o,
                lhsT=pwt[:, hg, :],
                rhs=aggT[:, hg, ch * 4 : ch * 4 + 4, :].rearrange("p a b -> p (a b)"),
                start=(hg == 0),
                stop=(hg == 1),
            )
        osb = pout.tile([64, 512], F32, tag="osb")
        nc.scalar.activation(out=osb, in_=ps_o, func=AF.Identity, bias=pwb, scale=1.0)
        # alternate issuing engine so output DMAs use two hardware queues
        eng = nc.sync if (y0 // 8 + ch) % 2 == 0 else nc.scalar
        eng.dma_start(out=out_d[:, ds(y0 * W + ch * 512, 512)], in_=osb)


def build_nc():
    nc = bacc.Bacc("TRN2", num_swdge_queues=4)
    x_d = nc.dram_tensor("x", [C, L], BF16, kind="ExternalInput").ap()
    sh_d = nc.dram_tensor("shiftm", [128, 256], BF16, kind="ExternalInput").ap()
    rhs0_d = nc.dram_tensor("rhs0", [C, 80], BF16, kind="ExternalInput").ap()
    pwt_d = nc.dram_tensor("pw_t2", [128, 2, 64], BF16, kind="ExternalInput").ap()
    gb_d = nc.dram_tensor("gate_b", [R], F32, kind="ExternalInput").ap()
    mb_d = nc.dram_tensor("geom_b", [12], F32, kind="ExternalInput").ap()
    pb_d = nc.dram_tensor("pw_b", [C], F32, kind="ExternalInput").ap()
    out_d = nc.dram_tensor("out", [C, L], F32, kind="ExternalOutput").ap()

    with tile.TileContext(nc) as tc:
        import contextlib

        with contextlib.ExitStack() as ctx:
            persist = ctx.enter_context(tc.tile_pool(name="persist", bufs=1))
            pools = {
                "geo": ctx.enter_context(tc.tile_pool(name="geo", bufs=2)),
                "mac": ctx.enter_context(tc.tile_pool(name="mac", bufs=2)),
                "out": ctx.enter_context(tc.tile_pool(name="out", bufs=2)),
                "psS": ctx.enter_context(
                    tc.tile_pool(name="psS", bufs=2, space="PSUM")
                ),
                "ps": ctx.enter_context(tc.tile_pool(name="ps", bufs=2, space="PSUM")),
                "psT": ctx.enter_context(
                    tc.tile_pool(name="psT", bufs=2, space="PSUM")
                ),
                "pso": ctx.enter_context(
                    tc.tile_pool(name="pso", bufs=2, space="PSUM")
                ),
            }

            x2 = persist.tile([128, 65 * W], BF16)
            rhs0 = persist.tile([128, 80], BF16)
            pwt = persist.tile([128, 2, 64], BF16)
            gbias = persist.tile([128, R], F32)
            mbias = persist.tile([128, 12], F32)
            pwb = persist.tile([64, 1], F32)
            ident = persist.tile([128, 128], BF16)
            make_identity(nc, ident)
            shiftm = persist.tile([128, 256], BF16)
            nc.sync.dma_start(out=shiftm, in_=sh_d)
            v = persist.tile([128, 64, 130], BF16)
            u = persist.tile([128, 64, 130], BF16)
            wd = persist.tile([128, 64, 130], BF16)
            gg = persist.tile([128, 16, 128], F32)
            G = [persist.tile([128, 4, 128], BF16, name=f"G{i}") for i in range(5)]

            cb = {}
            for cname, cval in [
                ("eps", EPS),
                ("mh2n", -2.0 * MIN_HYP),
                ("mh2p", 2.0 * MIN_HYP),
                ("mln2", -LN2),
            ]:
                t = persist.tile([128, 1], F32, tag="cb_" + cname, name="cb_" + cname)
                nc.gpsimd.memset(t, cval)
                cb[cname] = t

            CH = 65 * W // 8
            dma_engs = [nc.sync, nc.scalar, nc.gpsimd]
            for c8 in range(8):
                dma_engs[(2 * c8) % 3].dma_start(
                    out=x2[0:64, ds(c8 * CH, CH)], in_=x_d[:, ds(c8 * CH, CH)]
                )
                dma_engs[(2 * c8 + 1) % 3].dma_start(
                    out=x2[64:128, ds(c8 * CH, CH)],
                    in_=x_d[:, ds(63 * W + c8 * CH, CH)],
                )
            nc.sync.dma_start(out=rhs0[0:64, :], in_=rhs0_d)
            nc.sync.dma_start(out=rhs0[64:128, :], in_=rhs0_d)
            nc.sync.dma_start(out=pwt, in_=pwt_d)
            nc.gpsimd.dma_start(
                out=gbias,
                in_=bass.AP(tensor=gb_d.tensor, offset=0, ap=[[0, 128], [1, R]]),
            )
            nc.gpsimd.dma_start(
                out=mbias,
                in_=bass.AP(tensor=mb_d.tensor, offset=0, ap=[[0, 128], [1, 12]]),
            )
            nc.sync.dma_start(out=pwb, in_=pb_d.rearrange("(c o) -> c o", o=1))

            # y halos (disjoint from projection writes, so order-free)
            nc.gpsimd.memset(v[:, :, 0:1], 0.0)
            nc.gpsimd.memset(v[:, :, 129:130], 0.0)

            # software-pipelined per-quad emission keeps every engine's
            # in-order stream free of long cross-stage stalls
            _projection(nc, pools, x2, rhs0, v, gg, 0)
            _shift_uw(nc, pools, shiftm, v, u, wd, 0)
            _geometry(nc, pools, gg, gbias, mbias, cb, G, 0)
            _projection(nc, pools, x2, rhs0, v, gg, 1)
            _shift_uw(nc, pools, shiftm, v, u, wd, 1)
            for h in range(4):
                if h < 3:
                    chunks = [(32 * h, 16), (32 * h + 16, 16)]
                else:  # shorter tail drain on the last quad
                    chunks = [(96, 16), (112, 8), (120, 8)]
                for y0c, nyc in chunks:
                    _mac_band(
                        nc, pools, y0c, nyc, v, u, wd, G, ident, pwt, pwb, out_d
                    )
                if h + 2 <= 3:
                    _projection(nc, pools, x2, rhs0, v, gg, h + 2)
                    _shift_uw(nc, pools, shiftm, v, u, wd, h + 2)
                if h + 1 <= 3:
                    _geometry(nc, pools, gg, gbias, mbias, cb, G, h + 1)
    nc.compile()
    return nc


_NC_CACHE = {}


def _get_nc():
    if "nc" not in _NC_CACHE:
        _NC_CACHE["nc"] = build_nc()
    return _NC_CACHE["nc"]


def prep_core_inputs(inputs, core):
    x = np.ascontiguousarray(inputs["x"][core].reshape(C, L), dtype=np.float32)
    x = x.astype(ml_dtypes.bfloat16)
    value_w = inputs["value_w"].astype(np.float32)
    gate_w = inputs["gate_w"].astype(np.float32)
    geom_w = inputs["geom_w"].astype(np.float32)
    pw_w = inputs["pw_w"].astype(np.float32)
    rhs0 = np.concatenate([value_w.T, gate_w.T, geom_w.T], axis=1)  # [64, 80]
    pw_t2 = np.ascontiguousarray(
        pw_w.T.reshape(2, 128, 64).transpose(1, 0, 2)
    )  # [128, 2, 64]; pw_t2[p, h, o] = pw_w[o, h*128+p]
    tri = np.eye(128, k=1, dtype=np.float32) + np.eye(128, k=-1, dtype=np.float32)
    wdt = np.eye(128, k=1, dtype=np.float32) - np.eye(128, k=-1, dtype=np.float32)
    shiftm = np.concatenate([tri, wdt], axis=1)
    return {
        "x": x,
        "shiftm": shiftm.astype(ml_dtypes.bfloat16),
        "rhs0": np.ascontiguousarray(rhs0).astype(ml_dtypes.bfloat16),
        "pw_t2": pw_t2.astype(ml_dtypes.bfloat16),
        "gate_b": inputs["gate_b"].astype(np.float32),
        "geom_b": inputs["geom_b"].astype(np.float32),
        "pw_b": inputs["pw_b"].astype(np.float32),
    }


def kernel(**inputs):
    from concourse.bass_utils import run_bass_kernel_spmd

    nc = _get_nc()
    inputs = {k: np.asarray(v) for k, v in inputs.items()}
    in_maps = [prep_core_inputs(inputs, i) for i in range(B)]
    res = run_bass_kernel_spmd(nc, in_maps, core_ids=list(range(B)))
    out = np.stack([r["out"].reshape(C, H, W) for r in res.results])
    return out.astype(np.float32)


if __name__ == "__main__":
    nc = build_nc()
    print("built ok")


# revision 43
# speedup vs baseline: 1.1251x; 1.0011x over previous
"""AZConv2d fused anisotropic conv kernel for Trainium2 (Bass/Tile), v3.

Math (per pixel l, rule r):
  gate = gate_w@x + gate_b; mu = softmax_r(gate)
  v = value_w@x;  geom = geom_w@x + geom_b -> theta, raw_base, raw_hyper (4 each)
  base = softplus(rb)+1e-4; hyper = softplus(rh)+0.1
  iu2 = 1/(base*e^h)^2; is2 = 1/(base*e^-h)^2
  kern(dy,dx) = exp(-(a*dx^2 + b*dy^2 + c2*dx*dy)),
     a = ct^2*iu2 + st^2*is2, b = st^2*iu2+ct^2*is2, c2 = 2*ct*st*(iu2-is2)
  w = mu*kern / (sum_{r,s} mu*kern + 1e-6)
  agg[r,c] = sum_s w[r,s] * v[c, l+delta_s];  out = pw_w @ agg + pw_b

With V0=v, V1=u=v(l-1)+v(l+1), V2=v(l-W)+v(l+W), S=u(l-W)+u(l+W),
T=wd(l-W)-wd(l+W), wd=v(l-1)-v(l+1):
  agg_r = G0*V0 + G1*V1 + G2*V2 + G3*S + G4*T,
  G0=em', G1=em'A, G2=em'B, G3=em'AB*cosh(c2), G4=-em'AB*sinh(c2).

v3: whole-image persistent fields in c-major [128 x-pixels, 64 chan, y]
layout (y-minor keeps every MAC operand unit-stride innermost -> DVE 2x
mode). x-shifts via partition-shifted sbuf->sbuf DMA (no x1/xd ops, no
u/w projections, no halo recompute). MAC [r, c, y] iteration, PE
transposes + pointwise as in the baseline.

Sharding: data-parallel over batch, 1 image per NeuronCore (8 cores).
"""

import math
import sys

for p in ("/opt/trn_rl_repo",):
    if p not in sys.path:
        sys.path.insert(0, p)

import ml_dtypes
import numpy as np

import concourse.bass as bass
import concourse.mybir as mybir
import concourse.tile as tile
from concourse import bacc
from concourse.bass import ds
from concourse.masks import make_identity

F32 = mybir.dt.float32
BF16 = mybir.dt.bfloat16
AF = mybir.ActivationFunctionType
ALU = mybir.AluOpType

B = 8
C = 64
H = 128
W = 128
L = H * W
R = 4
BAND = 8
NBANDS = H // BAND  # 16
EPS = 1e-4
MIN_HYP = 0.1
LN2 = math.log(2.0)


def _projection(nc, pools, x2, rhs0, v, gg, q):
    """Row projections for quad q (rows 32q..32q+31) -> v slots (c-major)
    and gg [128, 16, 128] f32 (gate 0:4 | theta 4:8 | rbase 8:12 | rhyp 12:16).
    """
    pps = pools["ps"]
    half = q // 2
    p0 = 64 * half
    rowbase = 0 if half == 0 else 63

    def xrow(y):
        return x2[p0 : p0 + 64, ds((y - rowbase) * W, W)]

    base = 32 * q
    for g in range(7):  # 7 groups of 5 rows (last group 2) per quad
        ra = base + g * 5
        rb = min(ra + 5, base + 32)
        n = rb - ra
        ps = pps.tile([128, 5, 80], F32, tag="psv")
        for j in range(n):
            nc.tensor.matmul(
                out=ps[:, j, :],
                lhsT=xrow(ra + j),
                rhs=rhs0[p0 : p0 + 64, :],
                start=True,
                stop=True,
            )
        # transposed copies: psum [y, c] -> sbuf [c, y]
        nc.scalar.activation(
            out=v[:, :, ra + 1 : rb + 1],
            in_=ps[:, 0:n, 0:64].rearrange("p y c -> p c y"),
            func=AF.Copy,
        )
        nc.scalar.activation(
            out=gg[:, :, ra:rb],
            in_=ps[:, 0:n, 64:80].rearrange("p y f -> p f y"),
            func=AF.Copy,
        )


def _shift_uw(nc, pools, shiftm, v, u, wd, q):
    """u/wd (PE pair sum/diff over the pixel index) for quad q's slot range."""
    psS = pools["psS"]
    lo = 32 * q if q > 0 else 0
    hi = 32 * q + 32 if q < 3 else 130
    ybl = list(range(lo, hi, 8)) + [hi]
    for which, mat, dst in ((0, shiftm[:, 0:128], u), (1, shiftm[:, 128:256], wd)):
        for k in range(len(ybl) - 1):
            ya, yb = ybl[k], ybl[k + 1]
            n = yb - ya
            psu = psS.tile([128, 64, 8], F32, tag="psS")
            nc.tensor.matmul(
                out=psu[:, :, 0:n],
                lhsT=mat,
                rhs=v[:, :, ya:yb],
                start=True,
                stop=True,
            )
            nc.scalar.activation(
                out=dst[:, :, ya:yb], in_=psu[:, :, 0:n], func=AF.Copy
            )


def _geometry(nc, pools, gg, gbias, mbias, cb, G, h):
    """Coefficients G[0..4] (each [128, 4, 128] bf16, y-minor) for quad h
    (rows 32h..32h+31) from gg [128, 16, 128] f32."""
    pg = pools["geo"]
    Y = 32
    yc = slice(32 * h, 32 * h + 32)

    def gt(tag, nf=4, dt=F32):
        return pg.tile([128, nf, Y], dt, tag=tag, name=tag)

    gate = gg[:, 0:4, yc]
    theta = gg[:, 4:8, yc]

    nc.vector.tensor_tensor(
        out=gate, in0=gate, in1=gbias[:, :, None].broadcast_to([128, 4, Y]),
        op=ALU.add,
    )
    nc.vector.tensor_tensor(
        out=gg[:, 4:16, yc],
        in0=gg[:, 4:16, yc],
        in1=mbias[:, :, None].broadcast_to([128, 12, Y]),
        op=ALU.add,
    )

    # trig; ct/st packed as cst = [ct | st]
    cst = gt("cst", 8)
    ct, st = cst[:, 0:4, :], cst[:, 4:8, :]
    sh = gt("sh")
    nc.scalar.activation(out=st, in_=theta, func=AF.Sin)
    nc.scalar.activation(out=sh, in_=theta, func=AF.Sin, scale=0.5)
    sh2 = gt("sh2")
    nc.vector.tensor_tensor(out=sh2, in0=sh, in1=sh, op=ALU.mult)
    nc.vector.tensor_scalar(
        out=ct, in0=sh2, scalar1=-2.0, scalar2=1.0, op0=ALU.mult, op1=ALU.add
    )

    # softmax numerator without the max shift: the normalization cancels any
    # shift exactly, and |gate| stays O(1) here so exp cannot overflow
    em = gt("em")
    nc.scalar.activation(out=em, in_=gate, func=AF.Exp)

    # softplus(base|hyper) = ln(1 + exp(.)), then ln(base+eps)
    e8 = gt("e8", 8)
    nc.scalar.activation(out=e8, in_=gg[:, 8:16, yc], func=AF.Exp)
    sp8 = gt("sp8", 8)
    nc.scalar.activation(out=sp8, in_=e8, func=AF.Ln, bias=1.0)
    spb, sph = sp8[:, 0:4, :], sp8[:, 4:8, :]
    lb = gt("lb")
    nc.scalar.activation(out=lb, in_=spb, func=AF.Ln, bias=cb["eps"])

    tpl, tmi = gt("tpl"), gt("tmi")
    nc.vector.tensor_tensor(out=tpl, in0=sph, in1=lb, op=ALU.add)
    nc.vector.tensor_tensor(out=tmi, in0=sph, in1=lb, op=ALU.subtract)
    ii = gt("ii", 8)
    iu2, is2 = ii[:, 0:4, :], ii[:, 4:8, :]
    nc.scalar.activation(out=iu2, in_=tpl, func=AF.Exp, scale=-2.0, bias=cb["mh2n"])
    nc.scalar.activation(out=is2, in_=tmi, func=AF.Exp, scale=2.0, bias=cb["mh2p"])

    sq = gt("sq", 8)
    nc.vector.tensor_tensor(out=sq, in0=cst, in1=cst, op=ALU.mult)
    t12 = gt("t12", 8)
    nc.vector.tensor_tensor(out=t12, in0=sq, in1=ii, op=ALU.mult)
    av = gt("av")
    nc.vector.tensor_tensor(out=av, in0=t12[:, 0:4, :], in1=t12[:, 4:8, :], op=ALU.add)
    ssum, bv = gt("ssum"), gt("bv")
    nc.vector.tensor_tensor(out=ssum, in0=iu2, in1=is2, op=ALU.add)
    nc.vector.tensor_tensor(out=bv, in0=ssum, in1=av, op=ALU.subtract)

    cs, dio, c2h = gt("cs"), gt("dio"), gt("c2h")
    nc.vector.tensor_tensor(out=cs, in0=ct, in1=st, op=ALU.mult)
    nc.vector.tensor_tensor(out=dio, in0=iu2, in1=is2, op=ALU.subtract)
    nc.vector.tensor_tensor(out=c2h, in0=cs, in1=dio, op=ALU.mult)

    Aa, Bb = gt("Aa"), gt("Bb")
    nc.scalar.activation(out=Aa, in_=av, func=AF.Exp, scale=-1.0)
    nc.scalar.activation(out=Bb, in_=bv, func=AF.Exp, scale=-1.0)
    # Corner kernels, overflow-safe: a+b = ssum >= |c2|
    sc2, sc2m = gt("sc2"), gt("sc2m")
    nc.vector.scalar_tensor_tensor(
        out=sc2, in0=c2h, scalar=2.0, in1=ssum, op0=ALU.mult, op1=ALU.add
    )
    nc.vector.scalar_tensor_tensor(
        out=sc2m, in0=c2h, scalar=-2.0, in1=ssum, op0=ALU.mult, op1=ALU.add
    )
    ph, qh = gt("ph"), gt("qh")
    nc.scalar.activation(out=ph, in_=sc2, func=AF.Exp, scale=-1.0, bias=cb["mln2"])
    nc.scalar.activation(out=qh, in_=sc2m, func=AF.Exp, scale=-1.0, bias=cb["mln2"])
    ppqh, pmqh = gt("ppqh"), gt("pmqh")
    nc.vector.tensor_tensor(out=ppqh, in0=ph, in1=qh, op=ALU.add)
    nc.vector.tensor_tensor(out=pmqh, in0=ph, in1=qh, op=ALU.subtract)

    # Sk = 1 + 2(A+B) + 4*(P+Q)/2
    apb, w1, sk = gt("apb"), gt("w1"), gt("sk")
    nc.vector.tensor_tensor(out=apb, in0=Aa, in1=Bb, op=ALU.add)
    nc.vector.scalar_tensor_tensor(
        out=w1, in0=ppqh, scalar=2.0, in1=apb, op0=ALU.mult, op1=ALU.add
    )
    nc.vector.tensor_scalar(
        out=sk, in0=w1, scalar1=2.0, scalar2=1.0, op0=ALU.mult, op1=ALU.add
    )

    ws = gt("ws")
    nc.vector.tensor_tensor(out=ws, in0=em, in1=sk, op=ALU.mult)
    d01 = pg.tile([128, 1, Y], F32, tag="d01", name="d01")
    d23 = pg.tile([128, 1, Y], F32, tag="d23", name="d23")
    Dp = pg.tile([128, 1, Y], F32, tag="Dp", name="Dp")
    nc.vector.tensor_tensor(out=d01, in0=ws[:, 0:1, :], in1=ws[:, 1:2, :], op=ALU.add)
    nc.vector.tensor_tensor(out=d23, in0=ws[:, 2:3, :], in1=ws[:, 3:4, :], op=ALU.add)
    nc.vector.tensor_tensor(out=Dp, in0=d01, in1=d23, op=ALU.add)
    s01 = pg.tile([128, 1, Y], F32, tag="s01", name="s01")
    s23 = pg.tile([128, 1, Y], F32, tag="s23", name="s23")
    Smu = pg.tile([128, 1, Y], F32, tag="Smu", name="Smu")
    nc.vector.tensor_tensor(out=s01, in0=em[:, 0:1, :], in1=em[:, 1:2, :], op=ALU.add)
    nc.vector.tensor_tensor(out=s23, in0=em[:, 2:3, :], in1=em[:, 3:4, :], op=ALU.add)
    nc.vector.tensor_tensor(out=Smu, in0=s01, in1=s23, op=ALU.add)
    D2 = pg.tile([128, 1, Y], F32, tag="D2")
    nc.vector.scalar_tensor_tensor(
        out=D2, in0=Smu, scalar=1e-6, in1=Dp, op0=ALU.mult, op1=ALU.add
    )
    invD = pg.tile([128, 1, Y], F32, tag="invD")
    nc.vector.reciprocal(invD, D2)

    invDb = invD.broadcast_to([128, 4, Y])
    nc.vector.tensor_tensor(out=G[0][:, :, yc], in0=em, in1=invDb, op=ALU.mult)
    em2 = G[0][:, :, yc]
    nc.vector.tensor_tensor(out=G[1][:, :, yc], in0=em2, in1=Aa, op=ALU.mult)
    nc.vector.tensor_tensor(out=G[2][:, :, yc], in0=em2, in1=Bb, op=ALU.mult)
    nc.vector.tensor_tensor(out=G[3][:, :, yc], in0=em2, in1=ppqh, op=ALU.mult)
    nc.vector.tensor_tensor(out=G[4][:, :, yc], in0=em2, in1=pmqh, op=ALU.mult)


def _mac_band(nc, pools, y0, NY, v, u, wd, G, ident, pwt, pwb, out_d):
    """MAC + PE transpose + pointwise for rows y0..y0+NY-1."""
    pm, ppsT, ppso, pout = pools["mac"], pools["psT"], pools["pso"], pools["out"]
    sh = [128, R, 64, NY]  # [x, r, c, y]

    def bf(t, off):  # field [128, 64, 130] -> [r, c, y] view (bcast r)
        return t[:, None, :, y0 + off : y0 + off + NY].broadcast_to(sh)

    def bg(t):  # coeff [128, 4, 128] -> [r, c, y] view (bcast c)
        return t[:, :, None, y0 : y0 + NY].broadcast_to(sh)

    # v2/vs/vt for this band pair (c-major, y-minor views of whole fields)
    v2 = pm.tile([128, 64, NY], BF16, tag="v2")
    vs = pm.tile([128, 64, NY], BF16, tag="vs")
    vt = pm.tile([128, 64, NY], BF16, tag="vt")
    nc.vector.tensor_tensor(
        out=v2, in0=v[:, :, y0 : y0 + NY], in1=v[:, :, y0 + 2 : y0 + NY + 2],
        op=ALU.add,
    )
    nc.vector.tensor_tensor(
        out=vs, in0=u[:, :, y0 : y0 + NY], in1=u[:, :, y0 + 2 : y0 + NY + 2],
        op=ALU.add,
    )
    nc.vector.tensor_tensor(
        out=vt, in0=wd[:, :, y0 : y0 + NY], in1=wd[:, :, y0 + 2 : y0 + NY + 2],
        op=ALU.subtract,
    )

    def bb(t):  # band field [128, 64, 16] -> [r, c, y]
        return t[:, None, :, :].broadcast_to(sh)

    agg = pm.tile(sh, BF16, tag="agg")
    tA = pm.tile(sh, BF16, tag="tA")
    tB = pm.tile(sh, BF16, tag="tB")
    nc.vector.tensor_tensor(out=tA, in0=bf(v, 1), in1=bg(G[0]), op=ALU.mult)
    nc.vector.tensor_tensor(out=tB, in0=bf(u, 1), in1=bg(G[1]), op=ALU.mult)
    nc.vector.tensor_tensor(out=agg, in0=tA, in1=tB, op=ALU.add)
    nc.vector.tensor_tensor(out=tA, in0=bb(v2), in1=bg(G[2]), op=ALU.mult)
    nc.vector.tensor_tensor(out=tB, in0=bb(vs), in1=bg(G[3]), op=ALU.mult)
    nc.vector.tensor_tensor(out=agg, in0=agg, in1=tA, op=ALU.add)
    nc.vector.tensor_tensor(out=tA, in0=bb(vt), in1=bg(G[4]), op=ALU.mult)
    nc.vector.tensor_tensor(out=agg, in0=agg, in1=tB, op=ALU.add)
    nc.vector.tensor_tensor(out=agg, in0=agg, in1=tA, op=ALU.add)

    # transpose [pixel, (r c)] -> [(r c), pixel] via PE, half (128 rc) at a time
    aggT = pm.tile([128, 2, NY, 128], BF16, tag="aggT")
    for hg in range(2):
        for grp in range(NY // 4):
            psT = ppsT.tile([128, 4, 128], BF16, tag="psT")
            for yy in range(4):
                yr = grp * 4 + yy
                nc.tensor.transpose(
                    out=psT[:, yy, :],
                    in_=agg[:, 2 * hg : 2 * hg + 2, :, yr].rearrange(
                        "p a b -> p (a b)"
                    ),
                    identity=ident,
                )
            nc.scalar.activation(
                out=aggT[:, hg, grp * 4 : grp * 4 + 4, :], in_=psT, func=AF.Copy
            )

    # out = pw @ agg + pw_b (c-major), 512 pixels per psum chunk
    for ch in range(NY * W // 512):
        ps_o = ppso.tile([64, 512], F32, tag="pso")
        for hg in range(2):
            nc.tensor.matmul(
                out=ps_o,
                lhsT=pwt[:, hg, :],
                rhs=aggT[:, hg, ch * 4 : ch * 4 + 4, :].rearrange("p a b -> p (a b)"),
                start=(hg == 0),
                stop=(hg == 1),
            )
        osb = pout.tile([64, 512], F32, tag="osb")
        nc.scalar.activation(out=osb, in_=ps_o, func=AF.Identity, bias=pwb, scale=1.0)
        # alternate issuing engine so output DMAs use two hardware queues
        eng = nc.sync if (y0 // 8 + ch) % 2 == 0 else nc.scalar
        eng.dma_start(out=out_d[:, ds(y0 * W + ch * 512, 512)], in_=osb)


def build_nc():
    nc = bacc.Bacc("TRN2", num_swdge_queues=4)
    x_d = nc.dram_tensor("x", [C, L], BF16, kind="ExternalInput").ap()
    sh_d = nc.dram_tensor("shiftm", [128, 256], BF16, kind="ExternalInput").ap()
    rhs0_d = nc.dram_tensor("rhs0", [C, 80], BF16, kind="ExternalInput").ap()
    pwt_d = nc.dram_tensor("pw_t2", [128, 2, 64], BF16, kind="ExternalInput").ap()
    gb_d = nc.dram_tensor("gate_b", [R], F32, kind="ExternalInput").ap()
    mb_d = nc.dram_tensor("geom_b", [12], F32, kind="ExternalInput").ap()
    pb_d = nc.dram_tensor("pw_b", [C], F32, kind="ExternalInput").ap()
    out_d = nc.dram_tensor("out", [C, L], F32, kind="ExternalOutput").ap()

    with tile.TileContext(nc) as tc:
        import contextlib

        with contextlib.ExitStack() as ctx:
            persist = ctx.enter_context(tc.tile_pool(name="persist", bufs=1))
            pools = {
                "geo": ctx.enter_context(tc.tile_pool(name="geo", bufs=2)),
                "mac": ctx.enter_context(tc.tile_pool(name="mac", bufs=2)),
                "out": ctx.enter_context(tc.tile_pool(name="out", bufs=2)),
                "psS": ctx.enter_context(
                    tc.tile_pool(name="psS", bufs=2, space="PSUM")
                ),
                "ps": ctx.enter_context(tc.tile_pool(name="ps", bufs=2, space="PSUM")),
                "psT": ctx.enter_context(
                    tc.tile_pool(name="psT", bufs=2, space="PSUM")
                ),
                "pso": ctx.enter_context(
                    tc.tile_pool(name="pso", bufs=2, space="PSUM")
                ),
            }

            x2 = persist.tile([128, 65 * W], BF16)
            rhs0 = persist.tile([128, 80], BF16)
            pwt = persist.tile([128, 2, 64], BF16)
            gbias = persist.tile([128, R], F32)
            mbias = persist.tile([128, 12], F32)
            pwb = persist.tile([64, 1], F32)
            ident = persist.tile([128, 128], BF16)
            make_identity(nc, ident)
            shiftm = persist.tile([128, 256], BF16)
            nc.sync.dma_start(out=shiftm, in_=sh_d)
            v = persist.tile([128, 64, 130], BF16)
            u = persist.tile([128, 64, 130], BF16)
            wd = persist.tile([128, 64, 130], BF16)
            gg = persist.tile([128, 16, 128], F32)
            G = [persist.tile([128, 4, 128], BF16, name=f"G{i}") for i in range(5)]

            cb = {}
            for cname, cval in [
                ("eps", EPS),
                ("pi2", math.pi / 2),
                ("mh2n", -2.0 * MIN_HYP),
                ("mh2p", 2.0 * MIN_HYP),
                ("mln2", -LN2),
            ]:
                t = persist.tile([128, 1], F32, tag="cb_" + cname, name="cb_" + cname)
                nc.gpsimd.memset(t, cval)
                cb[cname] = t

            CH = 65 * W // 8
            dma_engs = [nc.sync, nc.scalar, nc.gpsimd]
            for c8 in range(8):
                dma_engs[(2 * c8) % 3].dma_start(
                    out=x2[0:64, ds(c8 * CH, CH)], in_=x_d[:, ds(c8 * CH, CH)]
                )
                dma_engs[(2 * c8 + 1) % 3].dma_start(
                    out=x2[64:128, ds(c8 * CH, CH)],
                    in_=x_d[:, ds(63 * W + c8 * CH, CH)],
                )
            nc.sync.dma_start(out=rhs0[0:64, :], in_=rhs0_d)
            nc.sync.dma_start(out=rhs0[64:128, :], in_=rhs0_d)
            nc.sync.dma_start(out=pwt, in_=pwt_d)
            nc.gpsimd.dma_start(
                out=gbias,
                in_=bass.AP(tensor=gb_d.tensor, offset=0, ap=[[0, 128], [1, R]]),
            )
            nc.gpsimd.dma_start(
                out=mbias,
                in_=bass.AP(tensor=mb_d.tensor, offset=0, ap=[[0, 128], [1, 12]]),
            )
            nc.sync.dma_start(out=pwb, in_=pb_d.rearrange("(c o) -> c o", o=1))

            # y halos (disjoint from projection writes, so order-free)
            nc.gpsimd.memset(v[:, :, 0:1], 0.0)
            nc.gpsimd.memset(v[:, :, 129:130], 0.0)

            # software-pipelined per-quad emission keeps every engine's
            # in-order stream free of long cross-stage stalls
            _projection(nc, pools, x2, rhs0, v, gg, 0)
            _shift_uw(nc, pools, shiftm, v, u, wd, 0)
            _geometry(nc, pools, gg, gbias, mbias, cb, G, 0)
            _projection(nc, pools, x2, rhs0, v, gg, 1)
            _shift_uw(nc, pools, shiftm, v, u, wd, 1)
            for h in range(4):
                if h < 3:
                    chunks = [(32 * h, 16), (32 * h + 16, 16)]
                else:  # shorter tail drain on the last quad
                    chunks = [(96, 16), (112, 8), (120, 8)]
                for y0c, nyc in chunks:
                    _mac_band(
                        nc, pools, y0c, nyc, v, u, wd, G, ident, pwt, pwb, out_d
                    )
                if h + 2 <= 3:
                    _projection(nc, pools, x2, rhs0, v, gg, h + 2)
                    _shift_uw(nc, pools, shiftm, v, u, wd, h + 2)
                if h + 1 <= 3:
                    _geometry(nc, pools, gg, gbias, mbias, cb, G, h + 1)
    nc.compile()
    return nc


_NC_CACHE = {}


def _get_nc():
    if "nc" not in _NC_CACHE:
        _NC_CACHE["nc"] = build_nc()
    return _NC_CACHE["nc"]


def prep_core_inputs(inputs, core):
    x = np.ascontiguousarray(inputs["x"][core].reshape(C, L), dtype=np.float32)
    x = x.astype(ml_dtypes.bfloat16)
    value_w = inputs["value_w"].astype(np.float32)
    gate_w = inputs["gate_w"].astype(np.float32)
    geom_w = inputs["geom_w"].astype(np.float32)
    pw_w = inputs["pw_w"].astype(np.float32)
    rhs0 = np.concatenate([value_w.T, gate_w.T, geom_w.T], axis=1)  # [64, 80]
    pw_t2 = np.ascontiguousarray(
        pw_w.T.reshape(2, 128, 64).transpose(1, 0, 2)
    )  # [128, 2, 64]; pw_t2[p, h, o] = pw_w[o, h*128+p]
    tri = np.eye(128, k=1, dtype=np.float32) + np.eye(128, k=-1, dtype=np.float32)
    wdt = np.eye(128, k=1, dtype=np.float32) - np.eye(128, k=-1, dtype=np.float32)
    shiftm = np.concatenate([tri, wdt], axis=1)
    return {
        "x": x,
        "shiftm": shiftm.astype(ml_dtypes.bfloat16),
        "rhs0": np.ascontiguousarray(rhs0).astype(ml_dtypes.bfloat16),
        "pw_t2": pw_t2.astype(ml_dtypes.bfloat16),
        "gate_b": inputs["gate_b"].astype(np.float32),
        "geom_b": inputs["geom_b"].astype(np.float32),
        "pw_b": inputs["pw_b"].astype(np.float32),
    }


def kernel(**inputs):
    from concourse.bass_utils import run_bass_kernel_spmd

    nc = _get_nc()
    inputs = {k: np.asarray(v) for k, v in inputs.items()}
    in_maps = [prep_core_inputs(inputs, i) for i in range(B)]
    res = run_bass_kernel_spmd(nc, in_maps, core_ids=list(range(B)))
    out = np.stack([r["out"].reshape(C, H, W) for r in res.results])
    return out.astype(np.float32)


if __name__ == "__main__":
    nc = build_nc()
    print("built ok")
